# revision 1
# baseline (speedup 1.0000x reference)
"""Trainium2 Bass kernel for nn_CTCAttentionDecoder.

12-layer transformer decoder (cross-attn over encoder memory + causal
self-attn with rotary embeddings + FFN) -> LM head -> masked NLL loss.

Parallelization: 8 NeuronCores = 4 pairs (one batch sample each); within a
pair, decoder tokens are split between the two cores (interleaved 128-token
chunks to balance causal attention work). K/V are computed redundantly on
both cores so the only per-layer communication is a single pair-AllGather of
the layer-normed activations (fp8). The LM head runs with full vocab on
each core for its own tokens; the host combines 8 per-token partial results
into the scalar loss.

Precision: residual stream and softmax/layernorm statistics in fp32;
attention scores/AV in bf16; all weight projections (QKVO, FFN, K/V-mem,
LM head) run in fp8 (e5m2 weights x e4m3 activations) with fp32 PSUM
accumulation, using DoubleRow 256-deep contraction where the stationary
operand's pair-step is 16B-aligned. Final rel err vs the fp32 reference is
~1e-3, against a 2e-2 tolerance.

Scheduling: engine queues execute in emission order, so the build software-
pipelines everything: self-attn projections are emitted as filler units
inside the ACT-bound cross-attention head loop, rope's swap-matmul and the
softmax normalization run 1-2 chunks/heads behind their producers, and
causally-dead score/exp/mask work is skipped via per-piece query ranges.
"""

import os

import numpy as np
import ml_dtypes

import concourse.bacc as bacc
import concourse.mybir as mybir
import concourse.tile as tile
from concourse.bass_utils import run_bass_kernel_spmd

F32 = mybir.dt.float32
BF16 = mybir.dt.bfloat16
F8E4 = mybir.dt.float8e4
F8E5 = mybir.dt.float8e5
DROW = mybir.MatmulPerfMode.DoubleRow
AF = mybir.ActivationFunctionType
ALU = mybir.AluOpType

N, S, T0, C, H, D, NLAYERS, V, FF = 4, 1024, 512, 1024, 16, 64, 12, 8192, 4096
T = T0 + 1  # 513
TO = 258  # own tokens per core (incl. pad columns)
TK = 514  # key slots: [c0 | c3 | c1 | c2 | t512 | pad]
STX, ETX = 3, 4
NEG = -1e30
KC_SLICES = [(0, 128), (128, 256), (256, 384), (384, 512), (512, 514)]

L = int(os.environ.get("K_LAYERS", str(NLAYERS)))

_prog_cache = {}


def _own_global_idx(r):
    """Global token index per own column; -1 for pad columns."""
    if r == 0:
        return np.concatenate([np.arange(0, 128), np.arange(384, 512), [-1, -1]])
    return np.concatenate([np.arange(128, 256), np.arange(256, 384), [512, -1]])


def _key_global_idx():
    """Global token index per key slot; -1 for the pad slot."""
    return np.concatenate(
        [np.arange(0, 128), np.arange(384, 512), np.arange(128, 256),
         np.arange(256, 384), [512, -1]]
    )


def _rope_tables(pos, rows=128):
    """cos/sin tables [rows, len(pos)]; row i uses theta_(i%32)."""
    th = (10000.0 ** (-2.0 * np.arange(32) / D))  # [32]
    ang = th[:, None] * np.maximum(pos, 0)[None, :].astype(np.float64)
    cos = np.cos(ang).astype(np.float32)
    sin = np.sin(ang).astype(np.float32)
    reps = rows // 32
    return np.tile(cos, (reps, 1)), np.tile(sin, (reps, 1))


def _self_masks(r):
    """Causal masks, multiplicative {0, 1}: [4, 128, TO] big chunks + [2, TO]
    tail. Applied to exp(scores) (p = exp(s) * m), so no NEG bias needed."""
    own = _own_global_idx(r)  # [TO]
    key = _key_global_idx()  # [TK]
    big = np.zeros((4, 128, TO), np.float32)
    for kc in range(4):
        kg = key[kc * 128:(kc + 1) * 128]
        big[kc] = (kg[:, None] <= own[None, :]).astype(np.float32)
    tail = (key[512:514, None] <= own[None, :]).astype(np.float32)
    tail[key[512:514] < 0, :] = 0.0  # pad key slot: never attended
    # Pad query columns: allow everything so rowsum > 0 (their output is
    # garbage-but-finite and never read; a fully-masked row gives 0/0 NaN
    # that would pollute real tokens through later layers).
    pad_q = own < 0
    big[:, :, pad_q] = 1.0
    tail[:, pad_q] = 1.0
    return big, tail


def _rope_perm():
    """Column permutation de-interleaving rotary pairs within each head."""
    p = np.arange(C).reshape(H, D)
    newd = np.concatenate([np.arange(0, D, 2), np.arange(1, D, 2)])
    return p[:, newd].reshape(-1)


def _pswap():
    """PT [128,128] with qswap = PT.T @ q: out[r]=-q[r+32], out[r+32]=q[r]."""
    PT = np.zeros((128, 128), np.float32)
    for b in range(0, 128, 64):
        for i in range(32):
            PT[b + 32 + i, b + i] = -1.0
            PT[b + i, b + 32 + i] = 1.0
    return PT


def _build_program(nlayers):
    nc = bacc.Bacc("TRN2", num_devices=8)

    def din(name, shape, dtype=BF16):
        return nc.dram_tensor(name, shape, dtype, kind="ExternalInput")

    t = {}
    t["y0"] = din("y0", [C, TO], F32)
    t["featT"] = din("featT", [C, S], F8E4)
    t["memmask"] = din("memmask", [S, 1], F32)
    t["mbig"] = din("mbig", [4, 128, TO], BF16)
    t["mtail"] = din("mtail", [2, TO], BF16)
    t["cosq"] = din("cosq", [128, TO], BF16)
    t["sinq"] = din("sinq", [128, TO], BF16)
    t["cosk"] = din("cosk", [128, TK], BF16)
    t["sink"] = din("sink", [128, TK], BF16)
    t["wtgt"] = din("wtgt", [C, TO])
    t["ptsw"] = din("ptsw", [128, 128])
    for nm in ["wqm", "wom", "wqt", "wkt", "wvt", "wot", "wkm", "wvm"]:
        t[nm] = din(nm, [nlayers, C, C], F8E5)
    t["w1"] = din("w1", [nlayers, C, FF], F8E5)
    t["w2"] = din("w2", [nlayers, FF, C], F8E5)
    t["wout"] = din("wout", [C, V], F8E5)

    t["out_se"] = nc.dram_tensor("out_se", [128, 3], F32, kind="ExternalOutput")
    t["out_tl"] = nc.dram_tensor("out_tl", [1, TO], F32, kind="ExternalOutput")

    t["ag_in"] = [nc.dram_tensor(f"agi{l}", [C, TO], F8E4, kind="Internal")
                  for l in range(nlayers)]
    t["ag_out"] = [nc.dram_tensor(f"ago{l}", [2, C, TO], F8E4, kind="Internal")
                   for l in range(nlayers)]
    t["RG"] = [[0, 1], [2, 3], [4, 5], [6, 7]]

    with tile.TileContext(nc) as tc:
        import contextlib
        with contextlib.ExitStack() as ctx:
            with nc.allow_low_precision(
                    reason="bf16 softmax denominators / LN stats are within "
                           "the 2e-2 output tolerance"):
                _build_body(nc, tc, nlayers, t, ctx)
    nc.finalize()
    return nc


def _build_body(nc, tc, nlayers, t, ctx):
    P = 128
    ec = ctx.enter_context
    persist = ec(tc.tile_pool(name="persist", bufs=1))
    wk = ec(tc.tile_pool(name="wk", bufs=2))     # [128,8,1024] bf16 weight mats
    w1p = ec(tc.tile_pool(name="w1p", bufs=3))   # [128,8,1024] bf16 ffn/wout
    pbf = ec(tc.tile_pool(name="pbf", bufs=4))   # [128,TO] bf16 exp'd scores
    scr = ec(tc.tile_pool(name="scr", bufs=3))   # [128,TO] fp32 scratch
    scrw = ec(tc.tile_pool(name="scrw", bufs=3))  # [128,512] scratch
    vrow = ec(tc.tile_pool(name="vrow", bufs=4))  # [1,TO] fp32 rows
    vrb = ec(tc.tile_pool(name="vrb", bufs=6))   # [1,TO] bf16 rows
    kvp = ec(tc.tile_pool(name="kvp", bufs=1))   # per-layer kv tiles
    hbf_p = ec(tc.tile_pool(name="hbf_p", bufs=8))  # ffn hidden tiles

    ps_a = ec(tc.tile_pool(name="ps_a", bufs=4, space="PSUM"))
    ps_b = ec(tc.tile_pool(name="ps_b", bufs=1, space="PSUM"))
    ps_o = ec(tc.tile_pool(name="ps_o", bufs=3, space="PSUM"))

    def pt3(nm, n, w, dtype):
        big = persist.tile([P, n, w], dtype, name=nm)
        return big, [big[:, i, :] for i in range(n)]

    yT3, yT = pt3("yT", 8, TO, F32)
    xn3, xn = pt3("xn", 8, TO, BF16)
    xn83, xn8 = pt3("xn8", 8, TO, F8E4)
    xh03, xh0 = pt3("xh0", 8, TO, F8E4)
    xh13, xh1 = pt3("xh1", 8, TO, F8E4)
    ftT3, ftT = pt3("ftT", 8, S, F8E4)
    qm3, qm = pt3("qm", 8, TO, BF16)
    qt3, qt = pt3("qt", 8, TO, BF16)
    osb3, osb = pt3("osb", 8, TO, BF16)   # cross attn o
    osb83, osb8 = pt3("osb8", 8, TO, F8E4)
    osb283, osb28 = pt3("osb28", 8, TO, F8E4)
    osb23, osb2 = pt3("osb2", 8, TO, BF16)  # self attn o
    mmask3, mmask = pt3("mmask", 8, 1, F32)
    mbig3, mbig_t = pt3("mbigt", 4, TO, BF16)
    mtail_t = persist.tile([2, TO], BF16, name="mtailt")
    cq = persist.tile([P, TO], BF16, name="cq")
    sq = persist.tile([P, TO], BF16, name="sq")
    ck = persist.tile([P, TK], BF16, name="ck")
    sk = persist.tile([P, TK], BF16, name="sk")
    wtg3, wtg = pt3("wtg", 8, TO, BF16)
    ptw = persist.tile([P, P], BF16, name="ptw")
    ones_k = persist.tile([P, 1], BF16, name="ones_k")
    ones_kf = persist.tile([P, 1], F32, name="ones_kf")
    ones_r64 = persist.tile([1, 64], BF16, name="ones_r64")
    ones_r128 = persist.tile([1, P], BF16, name="ones_r128")
    epsr = persist.tile([1, 1], F32, name="epsr")

    dma = nc.sync.dma_start
    wdma = nc.gpsimd.dma_start

    def chunked(dr, p=P):
        return dr.rearrange("(k p) x -> p k x", p=p)

    dma(yT3[:], chunked(t["y0"]))
    dma(ftT3[:], chunked(t["featT"]))
    dma(mmask3[:], chunked(t["memmask"]))
    dma(wtg3[:], chunked(t["wtgt"]))
    dma(mbig3[:], t["mbig"].rearrange("c p t -> p c t"))
    dma(mtail_t[:], t["mtail"][:, :])
    dma(cq[:], t["cosq"][:, :])
    dma(sq[:], t["sinq"][:, :])
    dma(ck[:], t["cosk"][:, :])
    dma(sk[:], t["sink"][:, :])
    dma(ptw[:], t["ptsw"][:, :])
    nc.vector.memset(ones_k[:], 1.0)
    nc.vector.memset(ones_kf[:], 1.0)
    nc.vector.memset(ones_r64[:], 1.0)
    nc.vector.memset(ones_r128[:], 1.0)
    nc.vector.memset(epsr[:], 1e-5)

    def layernorm(src_tiles, out_tiles, tag):
        """out (bf16) = (src - mu)/sqrt(var+eps); pad cols zeroed via pmask.

        Stat matmuls use bf16 operands (fp32 matmul is 4x slower); rsqrt is
        exp(-0.5*ln(var+eps)) so only the exp/ln ACT table set is needed.
        """
        sum1 = ps_o.tile([1, TO], F32, name="psb_o")
        sum2 = ps_o.tile([1, TO], F32, name="psb_o")
        # emit the casts first so the PE sum chain streams without waiting
        # per-k on ACT/DVE (engine queues execute in emission order)
        ysqs, ybfs = [], []
        for k in range(8):
            ysq = pbf.tile([P, TO], BF16, name="lnb", bufs=12)
            nc.scalar.square(out=ysq[:], in_=src_tiles[k][:])
            ysqs.append(ysq)
            ybf = pbf.tile([P, TO], BF16, name="lnb", bufs=12)
            nc.vector.tensor_copy(out=ybf[:], in_=src_tiles[k][:])
            ybfs.append(ybf)
        for k in range(8):
            nc.tensor.matmul(sum1[:], ones_k[:], ybfs[k][:],
                             start=(k == 0), stop=(k == 7))
            nc.tensor.matmul(sum2[:], ones_k[:], ysqs[k][:],
                             start=(k == 0), stop=(k == 7))
        mub_r = vrb.tile([1, TO], BF16, name="vrb")
        nc.scalar.activation(out=mub_r[:], in_=sum1[:], func=AF.Copy,
                             scale=1.0 / C)
        mub = ps_a.tile([P, 512], F32, name="psa")[:, :TO]
        nc.tensor.matmul(mub, ones_r128[:], mub_r[:], start=True, stop=True)
        mu = vrow.tile([1, TO], F32, name="vrow")
        nc.scalar.mul(out=mu[:], in_=sum1[:], mul=1.0 / C)
        musq = vrow.tile([1, TO], F32, name="vrow")
        nc.vector.tensor_mul(out=musq[:], in0=mu[:], in1=mu[:])
        var = vrow.tile([1, TO], F32, name="vrow")
        nc.scalar.activation(out=var[:], in_=sum2[:], func=AF.Copy, scale=1.0 / C)
        nc.vector.tensor_sub(out=var[:], in0=var[:], in1=musq[:])
        lnv = vrow.tile([1, TO], F32, name="vrow")
        nc.scalar.activation(out=lnv[:], in_=var[:], func=AF.Ln, bias=epsr[:])
        rinv_b = vrb.tile([1, TO], BF16, name="vrb")
        nc.scalar.activation(out=rinv_b[:], in_=lnv[:], func=AF.Exp, scale=-0.5)
        rsb = ps_a.tile([P, 512], F32, name="psa")[:, :TO]
        nc.tensor.matmul(rsb, ones_r128[:], rinv_b[:], start=True, stop=True)
        mub_s = scrw.tile([P, TO], BF16, name="lnbc", bufs=2)
        nc.vector.tensor_copy(out=mub_s[:], in_=mub)
        rsb_s = scrw.tile([P, TO], BF16, name="lnbc", bufs=2)
        nc.vector.tensor_copy(out=rsb_s[:], in_=rsb)
        for k in range(8):
            d = pbf.tile([P, TO], BF16, name="lnd", bufs=2)
            nc.vector.tensor_sub(out=d[:], in0=ybfs[k][:], in1=mub_s[:])
            nc.vector.tensor_tensor(out=out_tiles[k][:], in0=d[:], in1=rsb_s[:],
                                    op=ALU.mult)

    def load_w(dram, l, dt=BF16):
        # fp8 weight blocks ride in the w1p pool (same 16KB slot budget)
        wt = (wk.tile([P, 8, C], BF16, name="wk") if dt == BF16
              else w1p.tile([P, 8, C], dt, name="w1p"))
        wdma(wt[:], chunked(dram[l]))
        sl = [wt[:, k, :] for k in range(8)]
        sl.append(wt)
        return sl

    def proj_T(wtiles, rhs_tiles, consume):
        for m in range(8):
            pt = ps_a.tile([P, 512], F32, name="psa")[:, :TO]
            for k in range(8):
                nc.tensor.matmul(pt, wtiles[k][:, m * P:(m + 1) * P],
                                 rhs_tiles[k][:, :TO],
                                 start=(k == 0), stop=(k == 7))
            consume(pt, m)

    def rope1(pt, c0, c1, ctab):
        """Stage 1: pb = bf16(pt); a = pb*cos (all-bf16 DVE runs at 2x)."""
        w = c1 - c0
        pb = scrw.tile([P, TO], BF16, name="ropep")[:, :w]
        nc.vector.tensor_copy(out=pb, in_=pt)
        a = scrw.tile([P, TO], BF16, name="ropea")[:, :w]
        nc.vector.tensor_tensor(out=a, in0=pb, in1=ctab[:, c0:c1], op=ALU.mult)
        return a, pb

    def rope2(st, c0, c1, stab, out_bf):
        """Stage 2 (emit one chunk later): out = a + (PT.T@pb)*sin."""
        a, pb = st
        w = c1 - c0
        swp = ps_b.tile([P, TO], F32, name="psb")[:, :w]
        nc.tensor.matmul(swp, ptw[:], pb, start=True, stop=True)
        b = scrw.tile([P, TO], BF16, name="ropeb", bufs=2)[:, :w]
        nc.vector.tensor_tensor(out=b, in0=swp, in1=stab[:, c0:c1], op=ALU.mult)
        nc.vector.tensor_tensor(out=out_bf, in0=a, in1=b, op=ALU.add)

    def attention(q_tiles, k_tiles, v_tiles, pieces, o_tiles, o8_tiles,
                  mask_mode, tag, filler=None):
        """pieces: (kci, kr0, kr1, qc0, qc1) — key-slot rows x query cols.

        Piece 0 must span the full query range (its start=True write fills
        oaug's has_written bits for the whole accumulation region). Self
        masks are multiplicative {0,1} bf16 applied to exp(scores).

        `filler()` is invoked once per head to emit independent PE work into
        the queue — softmax (exp) throughput on the ACT engine is the
        bottleneck inside attention, so extra matmuls slot into PE gaps.
        """
        npc = len(pieces)

        rcs = {}

        def finalize(h, oaug):
            """Evacuate head h's oaug unnormalized (bf16) + its reciprocal
            softmax denominator; the normalizing multiply runs two heads
            later (norm) so the PE never waits on the DVE reciprocal."""
            ht, hb = h // 2, (h % 2) * 64
            nc.vector.tensor_copy(out=o_tiles[ht][hb:hb + 64, :],
                                  in_=oaug[0:64, :])
            rc = vrb.tile([1, TO], BF16, name="vrb")
            nc.vector.reciprocal(out=rc[:], in_=oaug[64:65, :])
            rcs[h] = rc

        def norm(m):
            rb = ps_a.tile([P, 512], F32, name="psa")[:, :TO]
            nc.tensor.matmul(rb[0:64, :], ones_r64[:], rcs.pop(2 * m)[:],
                             start=True, stop=True)
            nc.tensor.matmul(rb[64:128, :], ones_r64[:], rcs.pop(2 * m + 1)[:],
                             start=True, stop=True)
            nc.vector.tensor_tensor(out=o8_tiles[m][:], in0=o_tiles[m][:],
                                    in1=rb, op=ALU.mult)

        pending = []  # (h, oaug) of recent heads, not yet evacuated
        normq = []
        for h in range(H):
            ht, hb = h // 2, (h % 2) * 64
            oaug = ps_o.tile([65, TO], F32, name="psb_o")
            ps = []
            for j, piece in enumerate(pieces):
                (kci, kr0, kr1, qc0, qc1), mrng = piece[:5], piece[5:]
                rows = kr1 - kr0
                w = qc1 - qc0
                sp = ps_a.tile([P, 512], F32, name="psa")[:rows, :w]
                nc.tensor.matmul(
                    sp, k_tiles[ht][hb:hb + 64, kr0:kr1],
                    q_tiles[ht][hb:hb + 64, qc0:qc1],
                    start=True, stop=True, tile_position=(hb, 0),
                )
                p = pbf.tile([P, TO], BF16, name="pbf")[:rows, :w]
                if mask_mode == "mem":
                    nc.scalar.activation(out=p, in_=sp, func=AF.Exp,
                                         bias=mmask[kci][:rows, :])
                else:
                    nc.scalar.activation(out=p, in_=sp, func=AF.Exp)
                    mc0, mc1 = mrng
                    mt = (mbig_t[kci][:rows, mc0:mc1] if kci < 4
                          else mtail_t[0:rows, mc0:mc1])
                    pm = p[:, mc0 - qc0:mc1 - qc0]
                    nc.vector.tensor_tensor(out=pm, in0=pm, in1=mt,
                                            op=ALU.mult)
                ps.append((j, kci, rows, qc0, qc1, p))
            if normq:
                norm(normq.pop(0))
            if len(pending) >= 1:
                finalize(*pending.pop(0))
                if h - 1 >= 1 and (h - 1) % 2 == 1:
                    normq.append((h - 1) // 2)
            for (j, kci, rows, qc0, qc1, p) in ps:
                nc.tensor.matmul(oaug[:, qc0:qc1],
                                 v_tiles[kci][:rows, h * 65:(h + 1) * 65],
                                 p, start=(j == 0), stop=(j == npc - 1))
            pending.append((h, oaug))
            if filler is not None:
                filler()
        for pd in pending:
            finalize(*pd)
        for m in normq:
            norm(m)
        for m in range(8):
            if 2 * m in rcs:
                norm(m)

    def make_v(w3, src3d, src_sel, vtiles):
        """v tiles [rows, 16*65] bf16 with a ones column per head (fp8
        DoubleRow: both ftT and wv are fp8, contraction 256/pass)."""
        for i, (c0, c1) in enumerate(src_sel):
            rows = c1 - c0
            vt = vtiles[i]
            vt3 = vt.rearrange("r (h x) -> r h x", x=65)
            for half in range(2):
                pv = ps_a.tile([P, 512], F32, name="psa")[:rows, :]
                for c in range(4):
                    nc.tensor.matmul(
                        pv, src3d[:, 2 * c:2 * c + 2, c0:c1],
                        w3[:, 2 * c:2 * c + 2,
                           half * 512:(half + 1) * 512],
                        start=(c == 0), stop=(c == 3), perf_mode=DROW)
                src3 = pv.rearrange("r (h d) -> r h d", d=64)
                nc.vector.tensor_copy(
                    out=vt3[:rows, half * 8:(half + 1) * 8, 0:64], in_=src3)
            nc.vector.memset(vt3[:rows, :, 64:65], 1.0)

    # ================= layers =================
    for l in range(nlayers):
        layernorm(yT, xn8, f"ln1_{l}")
        dma(chunked(t["ag_in"][l]), xn83[:])
        nc.gpsimd.collective_compute(
            "AllGather", ALU.bypass, replica_groups=t["RG"],
            ins=[t["ag_in"][l][:]], outs=[t["ag_out"][l][:]],
        )
        dma(xh03[:], chunked(t["ag_out"][l][0]))
        dma(xh13[:], chunked(t["ag_out"][l][1]))

        # ---- cross attention (overlaps the AllGather) ----
        wq = load_w(t["wqm"], l, F8E5)[8]
        for m in range(8):
            pt = ps_a.tile([P, 512], F32, name="psa")[:, :TO]
            for c in range(4):
                nc.tensor.matmul(pt, wq[:, 2 * c:2 * c + 2, m * P:(m + 1) * P],
                                 xn83[:, 2 * c:2 * c + 2, :],
                                 start=(c == 0), stop=(c == 3), perf_mode=DROW)
            nc.vector.tensor_copy(out=qm[m][:], in_=pt)
        wv = load_w(t["wvm"], l, F8E5)[8]
        vm = [kvp.tile([P, 16 * 65], BF16, name=f"vm{i}") for i in range(8)]
        make_v(wv, ftT3, [(i * P, (i + 1) * P) for i in range(8)], vm)
        wkm_t = load_w(t["wkm"], l, F8E5)[8]
        km = [kvp.tile([P, S], BF16, name=f"km{i}") for i in range(8)]
        for half in range(2):
            for m in range(8):
                pt = ps_a.tile([P, 512], F32, name="psa")
                for c in range(4):
                    nc.tensor.matmul(
                        pt[:], wkm_t[:, 2 * c:2 * c + 2, m * P:(m + 1) * P],
                        ftT3[:, 2 * c:2 * c + 2,
                             half * 512:(half + 1) * 512],
                        start=(c == 0), stop=(c == 3), perf_mode=DROW)
                nc.vector.tensor_copy(
                    out=km[m][:, half * 512:(half + 1) * 512], in_=pt[:])
        # ---- self-attn projections, emitted as filler units inside the
        # (ACT-bound) cross-attention head loop so the PE queue never drains.
        rpend = []

        def flush_rope(limit=2):
            while len(rpend) > limit:
                st, c0, c1, stab, dst = rpend.pop(0)
                rope2(st, c0, c1, stab, dst)

        wcur = {}
        kt = [kvp.tile([P, TK], BF16, name=f"kt{i}") for i in range(8)]
        vt_tiles = [kvp.tile([P, 16 * 65], BF16, name=f"vt{i}") for i in range(4)]
        vt_tiles.append(kvp.tile([2, 16 * 65], BF16, name="vt4"))
        units = []

        def u_load(key, dram):
            def f():
                wcur[key] = load_w(dram, l, F8E5)[8]
            return f

        def u_qt(m):
            def f():
                w3 = wcur["qt"]
                pt = ps_a.tile([P, 512], F32, name="psa")[:, :TO]
                for c in range(4):
                    nc.tensor.matmul(
                        pt, w3[:, 2 * c:2 * c + 2, m * P:(m + 1) * P],
                        xn83[:, 2 * c:2 * c + 2, :],
                        start=(c == 0), stop=(c == 3), perf_mode=DROW)
                rpend.append((rope1(pt, 0, TO, cq), 0, TO, sq, qt[m][:]))
                flush_rope()
            return f

        def u_kt(m, src3d, c0, cw):
            def f():
                w3 = wcur["kt"]
                pt = ps_a.tile([P, 512], F32, name="psa")[:, :cw]
                for c in range(4):
                    nc.tensor.matmul(
                        pt, w3[:, 2 * c:2 * c + 2, m * P:(m + 1) * P],
                        src3d[:, 2 * c:2 * c + 2, :cw],
                        start=(c == 0), stop=(c == 3), perf_mode=DROW)
                rpend.append((rope1(pt, c0, c0 + cw, ck), c0, c0 + cw, sk,
                              kt[m][:, c0:c0 + cw]))
                flush_rope()
            return f

        def u_vt(i, src3d, c0, c1, half):
            def f():
                rows = c1 - c0
                w3 = wcur["vt"]
                vt3 = vt_tiles[i].rearrange("r (h x) -> r h x", x=65)
                pv = ps_a.tile([P, 512], F32, name="psa")[:rows, :]
                for k in range(8):
                    nc.tensor.matmul(
                        pv, src3d[:, k, c0:c1],
                        w3[:, k, half * 512:(half + 1) * 512],
                        start=(k == 0), stop=(k == 7))
                src3 = pv.rearrange("r (h d) -> r h d", d=64)
                nc.vector.tensor_copy(
                    out=vt3[:rows, half * 8:(half + 1) * 8, 0:64], in_=src3)
                if half == 1:
                    nc.vector.memset(vt3[:rows, :, 64:65], 1.0)
            return f

        units.append(u_load("qt", t["wqt"]))
        for m in range(8):
            units.append(u_qt(m))
        units.append(u_load("kt", t["wkt"]))
        for m in range(8):
            for (src3d, c0, cw) in [(xh03, 0, 256), (xh13, 256, TO)]:
                units.append(u_kt(m, src3d, c0, cw))
        units.append(u_load("vt", t["wvt"]))
        for i, (src3d, c0, c1) in enumerate(
                [(xh03, 0, 128), (xh03, 128, 256), (xh13, 0, 128),
                 (xh13, 128, 256), (xh13, 256, TO)]):
            for half in range(2):
                units.append(u_vt(i, src3d, c0, c1, half))
        units.append(lambda: flush_rope(0))

        def cross_filler(n=2):
            for _ in range(n):
                if units:
                    units.pop(0)()

        attention(qm, km, vm,
                  [(i, i * P, (i + 1) * P, 0, TO) for i in range(8)],
                  osb, osb8, "mem", f"x{l}", filler=cross_filler)
        while units:
            units.pop(0)()

        # causal-trimmed pieces: key slots [r0A|r0B|r1A|r1B|tail] vs own
        # query cols; slot 0 must span the full query range (start=True).
        # mask multiplies cover only the (mc0, mc1) sub-ranges with zeros.
        attention(qt, kt, vt_tiles,
                  [(0, 0, 128, 0, TO, 0, 128), (1, 128, 256, 128, TO, 128, TO),
                   (2, 256, 384, 0, TO, 0, 128), (3, 384, 512, 128, TO, 128, TO),
                   (4, 512, 514, 256, TO, 256, TO)],
                  osb2, osb28, "self", f"s{l}")

        # ---- combined output projections (DVE: single add into y) ----
        wo1 = load_w(t["wom"], l, F8E5)[8]
        wo2 = load_w(t["wot"], l, F8E5)[8]
        for m in range(8):
            pt = ps_a.tile([P, 512], F32, name="psa")[:, :TO]
            for c in range(4):
                nc.tensor.matmul(
                    pt, wo1[:, 2 * c:2 * c + 2, m * P:(m + 1) * P],
                    osb83[:, 2 * c:2 * c + 2, :],
                    start=(c == 0), stop=False, perf_mode=DROW)
            for c in range(4):
                nc.tensor.matmul(
                    pt, wo2[:, 2 * c:2 * c + 2, m * P:(m + 1) * P],
                    osb283[:, 2 * c:2 * c + 2, :],
                    start=False, stop=(c == 3), perf_mode=DROW)
            nc.vector.tensor_tensor(out=yT[m][:], in0=pt, in1=yT[m][:],
                                    op=ALU.add)

        # ---- FFN (fp8 DoubleRow: h = gelu(xn8 @ W1sub), y += W2sub^T h) ----
        layernorm(yT, xn8, f"ln2_{l}")
        for sub in range(4):
            w1b = w1p.tile([P, 8, 1024], F8E5, name="w1p")
            wdma(w1b[:], chunked(t["w1"][l, :, sub * 1024:(sub + 1) * 1024]))
            hsub = []
            for mc in range(4):
                hp = hbf_p.tile([P, 2, TO], F8E4, name="hbf")
                for j in range(2):
                    m = 2 * mc + j
                    pt = ps_a.tile([P, 512], F32, name="psa")[:, :TO]
                    for c in range(4):
                        nc.tensor.matmul(
                            pt, w1b[:, 2 * c:2 * c + 2, m * P:(m + 1) * P],
                            xn83[:, 2 * c:2 * c + 2, :],
                            start=(c == 0), stop=(c == 3), perf_mode=DROW)
                    nc.scalar.activation(out=hp[:, j, :], in_=pt, func=AF.Gelu)
                hsub.append(hp)
            w2b = w1p.tile([P, 8, 1024], F8E5, name="w1p")
            wdma(w2b[:], chunked(t["w2"][l, sub * 1024:(sub + 1) * 1024, :]))
            for m in range(8):
                pt = ps_a.tile([P, 512], F32, name="psa")[:, :TO]
                for c in range(4):
                    nc.tensor.matmul(
                        pt, w2b[:, 2 * c:2 * c + 2, m * P:(m + 1) * P],
                        hsub[c][:, :, :],
                        start=(c == 0), stop=(c == 3), perf_mode=DROW)
                nc.vector.tensor_tensor(out=yT[m][:], in0=pt, in1=yT[m][:],
                                        op=ALU.add)

    # ================= head =================
    layernorm(yT, xn, "lnf")
    for k in range(8):
        nc.scalar.activation(out=xn8[k][:], in_=xn[k][:], func=AF.Copy)
    se = persist.tile([P, 3], F32, name="se")
    separt = persist.tile([P, 3, 16], F32, name="separt")
    nc.vector.memset(se[:], 0.0)
    TCS = [(0, 128), (128, 256), (256, TO)]
    for nvp in range(8):
        wb = w1p.tile([P, 8, 1024], F8E5, name="w1p")
        wdma(wb[:], chunked(t["wout"][:, nvp * 1024:(nvp + 1) * 1024]))
        for half in range(2):
            nv = nvp * 2 + half
            for tcn, (c0, c1) in enumerate(TCS):
                rows = c1 - c0
                pt = ps_a.tile([P, 512], F32, name="psa")[:rows, :]
                for k in range(8):
                    nc.tensor.matmul(
                        pt, xn83[:, k, c0:c1],
                        wb[:, k, half * 512:(half + 1) * 512],
                        start=(k == 0), stop=(k == 7))
                esc = scrw.tile([P, 512], BF16, name="scrwb")[:rows, :]
                nc.scalar.activation(out=esc, in_=pt, func=AF.Exp,
                                     accum_out=separt[0:rows, tcn, nv:nv + 1])
    for tcn, (c0, c1) in enumerate(TCS):
        rows = c1 - c0
        nc.vector.tensor_reduce(out=se[0:rows, tcn:tcn + 1],
                                in_=separt[0:rows, tcn, :],
                                axis=mybir.AxisListType.X, op=ALU.add)
    dma(t["out_se"][:, :], se[:])

    # target logit: tl = sum_c wtgt * yf (bf16-rounded, matching logits path)
    tlacc = scr.tile([P, TO], F32, name="tlacc", bufs=1)
    for k in range(8):
        xf = scr.tile([P, TO], F32, name="scr")
        nc.vector.tensor_tensor(out=xf[:], in0=xn[k][:], in1=wtg[k][:],
                                op=ALU.mult)
        if k == 0:
            nc.vector.tensor_copy(out=tlacc[:], in_=xf[:])
        else:
            nc.vector.tensor_add(out=tlacc[:], in0=tlacc[:], in1=xf[:])
    tlp = ps_o.tile([1, TO], F32, name="psb_o")
    nc.tensor.matmul(tlp[:], ones_kf[:], tlacc[:], start=True, stop=True)
    tls = vrow.tile([1, TO], F32, name="vrow")
    nc.scalar.activation(out=tls[:], in_=tlp[:], func=AF.Copy)
    dma(t["out_tl"][:, :], tls[:])


# ======================= host side =======================

def _host_inputs(features, targets, input_lengths, target_lengths, wte, ln1_w,
                 Wq_m, Wk_m, Wv_m, Wo_m, Wq_t, Wk_t, Wv_t, Wo_t, ln2_w, W1,
                 W2, lnf_w, Wout, nlayers):
    bf = ml_dtypes.bfloat16
    f32 = np.float32
    features = np.asarray(features, f32)
    targets = np.asarray(targets).astype(np.int64)
    input_lengths = np.asarray(input_lengths).astype(np.int64)
    target_lengths = np.asarray(target_lengths).astype(np.int64)
    ln1_w = np.asarray(ln1_w, f32)
    ln2_w = np.asarray(ln2_w, f32)
    lnf_w = np.asarray(lnf_w, f32)

    n = targets.shape[0]
    prompt = np.concatenate(
        [np.full((n, 1), STX, np.int64), targets], axis=1)  # [N, T]
    tgt = np.concatenate([targets, np.zeros((n, 1), np.int64)], axis=1)
    tgt[np.arange(n), target_lengths] = ETX

    perm = _rope_perm()
    pos_k = _key_global_idx()
    f8 = 1.0 / np.sqrt(np.float32(D))

    def cast(x):
        return np.ascontiguousarray(np.asarray(x, f32)).astype(bf)

    e4 = ml_dtypes.float8_e4m3
    e5 = ml_dtypes.float8_e5m2

    def cast8(x):
        return np.ascontiguousarray(np.asarray(x, f32)).astype(e5)

    sharedw = {
        "wqm": cast8(np.asarray(Wq_m, f32)[:nlayers] * ln1_w[:nlayers, :, None] * f8),
        "wkm": cast8(np.asarray(Wk_m, f32)[:nlayers]),
        "wvm": cast8(np.asarray(Wv_m, f32)[:nlayers]),
        "wom": cast8(np.asarray(Wo_m, f32)[:nlayers]),
        "wqt": cast8((np.asarray(Wq_t, f32)[:nlayers] * ln1_w[:nlayers, :, None]
                      * f8)[:, :, perm]),
        "wkt": cast8((np.asarray(Wk_t, f32)[:nlayers]
                      * ln1_w[:nlayers, :, None])[:, :, perm]),
        "wvt": cast8(np.asarray(Wv_t, f32)[:nlayers] * ln1_w[:nlayers, :, None]),
        "wot": cast8(np.asarray(Wo_t, f32)[:nlayers]),
        "w1": cast8(np.asarray(W1, f32)[:nlayers] * ln2_w[:nlayers, :, None]),
        "w2": cast8(np.asarray(W2, f32)[:nlayers]),
        "ptsw": _pswap().astype(bf),
    }
    wout_f = np.asarray(Wout, f32) * lnf_w[:, None]
    wout_bf = (np.asarray(Wout, f32) * lnf_w[:, None]).astype(e5)
    sharedw["wout"] = wout_bf

    y0_all = np.asarray(wte, f32)[prompt]  # [N, T, C]

    in_maps, meta = [], []
    for core in range(8):
        nb, r = core // 2, core % 2
        own = _own_global_idx(r)
        ownpos = np.maximum(own, 0)
        y0T = np.where(own[None, :] >= 0, y0_all[nb][ownpos].T, 0.0).astype(f32)
        cosq_, sinq_ = _rope_tables(own)
        cosk_, sink_ = _rope_tables(pos_k)
        mbig, mtail = _self_masks(r)
        memmask = np.where(np.arange(S) < input_lengths[nb], 0.0,
                           NEG).astype(f32)[:, None]
        padmask = (own >= 0).astype(f32)[None, :]
        wtgt = np.where(own[None, :] >= 0,
                        wout_bf.astype(f32)[:, tgt[nb][ownpos]], 0.0).astype(bf)
        im = {
            "y0": y0T,
            "featT": np.ascontiguousarray(features[nb].T).astype(e4),
            "memmask": memmask,
            "mbig": mbig.astype(bf), "mtail": mtail.astype(bf),
            "padmask": padmask,
            "cosq": cosq_.astype(bf), "sinq": sinq_.astype(bf),
            "cosk": cosk_.astype(bf), "sink": sink_.astype(bf),
            "wtgt": wtgt,
        }
        im.update(sharedw)
        in_maps.append(im)
        valid = np.where(own >= 0, (tgt[nb][ownpos] != 0), False)
        meta.append((nb, own, valid))
    return in_maps, meta


def kernel(features, targets, input_lengths, target_lengths, wte, ln1_w,
           Wq_m, Wk_m, Wv_m, Wo_m, Wq_t, Wk_t, Wv_t, Wo_t, ln2_w, W1, W2,
           lnf_w, Wout):
    nlayers = L
    if nlayers not in _prog_cache:
        _prog_cache[nlayers] = _build_program(nlayers)
    nc = _prog_cache[nlayers]

    in_maps, meta = _host_inputs(
        features, targets, input_lengths, target_lengths, wte, ln1_w,
        Wq_m, Wk_m, Wv_m, Wo_m, Wq_t, Wk_t, Wv_t, Wo_t, ln2_w, W1, W2,
        lnf_w, Wout, nlayers)

    res = run_bass_kernel_spmd(nc, in_maps, core_ids=list(range(8)))
    globals()["LAST_RESULTS"] = res

    num, den = 0.0, 0.0
    for core in range(8):
        r = res.results[core]
        _, own, valid = meta[core]
        se = r["out_se"]
        tl = r["out_tl"][0]
        sumexp = np.concatenate([se[:, 0], se[:, 1], se[:2, 2]])
        nll = np.log(np.maximum(sumexp, 1e-300)) - tl
        num += float(np.sum(nll[valid]))
        den += float(np.sum(valid))
    return np.float32(num / max(den, 1.0))



# revision 16
# speedup vs baseline: 1.1531x; 1.1531x over previous
"""Trainium2 Bass kernel for nn_CTCAttentionDecoder.

12-layer transformer decoder (cross-attn over encoder memory + causal
self-attn with rotary embeddings + FFN) -> LM head -> masked NLL loss.

Parallelization: 8 NeuronCores = 4 pairs (one batch sample each); within a
pair, decoder tokens are split between the two cores (interleaved 128-token
chunks to balance causal attention work). K/V are computed redundantly on
both cores so the only per-layer communication is a single pair-AllGather of
the layer-normed activations (fp8). The LM head runs with full vocab on
each core for its own tokens; the host combines 8 per-token partial results
into the scalar loss.

Precision: residual stream and softmax/layernorm statistics in fp32;
attention scores/AV in bf16; all weight projections (QKVO, FFN, K/V-mem,
LM head) run in fp8 (e5m2 weights x e4m3 activations) with fp32 PSUM
accumulation, using DoubleRow 256-deep contraction where the stationary
operand's pair-step is 16B-aligned. Final rel err vs the fp32 reference is
~1e-3, against a 2e-2 tolerance.

Scheduling: engine queues execute in emission order, so the build software-
pipelines everything: self-attn projections are emitted as filler units
inside the ACT-bound cross-attention head loop, rope's swap-matmul and the
softmax normalization run 1-2 chunks/heads behind their producers, and
causally-dead score/exp/mask work is skipped via per-piece query ranges.
"""

import os

import numpy as np
import ml_dtypes

import concourse.bacc as bacc
import concourse.mybir as mybir
import concourse.tile as tile
from concourse.bass_utils import run_bass_kernel_spmd

F32 = mybir.dt.float32
BF16 = mybir.dt.bfloat16
F8E4 = mybir.dt.float8e4
F8E5 = mybir.dt.float8e5
DROW = mybir.MatmulPerfMode.DoubleRow
AF = mybir.ActivationFunctionType
ALU = mybir.AluOpType

N, S, T0, C, H, D, NLAYERS, V, FF = 4, 1024, 512, 1024, 16, 64, 12, 8192, 4096
T = T0 + 1  # 513
TO = 258  # own tokens per core (incl. pad columns)
TOP = 272  # padded xh stride (16B-aligned pair-step for DoubleRow)
TK = 514  # key slots: [c0 | c3 | c1 | c2 | t512 | pad]
STX, ETX = 3, 4
NEG = -1e30

L = int(os.environ.get("K_LAYERS", str(NLAYERS)))

_prog_cache = {}


def _own_global_idx(r):
    """Global token index per own column; -1 for pad columns."""
    if r == 0:
        return np.concatenate([np.arange(0, 128), np.arange(384, 512), [-1, -1]])
    return np.concatenate([np.arange(128, 256), np.arange(256, 384), [512, -1]])


def _key_global_idx():
    """Global token index per key slot; -1 for the pad slot."""
    return np.concatenate(
        [np.arange(0, 128), np.arange(384, 512), np.arange(128, 256),
         np.arange(256, 384), [512, -1]]
    )


def _rope_tables(pos, rows=128):
    """cos/sin tables [rows, len(pos)]; row i uses theta_(i%32)."""
    th = (10000.0 ** (-2.0 * np.arange(32) / D))  # [32]
    ang = th[:, None] * np.maximum(pos, 0)[None, :].astype(np.float64)
    cos = np.cos(ang).astype(np.float32)
    sin = np.sin(ang).astype(np.float32)
    reps = rows // 32
    return np.tile(cos, (reps, 1)), np.tile(sin, (reps, 1))


def _self_masks(r):
    """Causal masks, multiplicative {0, 1}: [4, 128, TO] big chunks + [2, TO]
    tail. Applied to exp(scores) (p = exp(s) * m), so no NEG bias needed."""
    own = _own_global_idx(r)  # [TO]
    key = _key_global_idx()  # [TK]
    big = np.zeros((4, 128, TO), np.float32)
    for kc in range(4):
        kg = key[kc * 128:(kc + 1) * 128]
        big[kc] = (kg[:, None] <= own[None, :]).astype(np.float32)
    tail = (key[512:514, None] <= own[None, :]).astype(np.float32)
    tail[key[512:514] < 0, :] = 0.0  # pad key slot: never attended
    # Pad query columns: allow everything so rowsum > 0 (their output is
    # garbage-but-finite and never read; a fully-masked row gives 0/0 NaN
    # that would pollute real tokens through later layers).
    pad_q = own < 0
    big[:, :, pad_q] = 1.0
    tail[:, pad_q] = 1.0
    return big, tail


def _rope_perm():
    """Column permutation de-interleaving rotary pairs within each head."""
    p = np.arange(C).reshape(H, D)
    newd = np.concatenate([np.arange(0, D, 2), np.arange(1, D, 2)])
    return p[:, newd].reshape(-1)


def _pswap():
    """PT [128,128] with qswap = PT.T @ q: out[r]=-q[r+32], out[r+32]=q[r]."""
    PT = np.zeros((128, 128), np.float32)
    for b in range(0, 128, 64):
        for i in range(32):
            PT[b + 32 + i, b + i] = -1.0
            PT[b + i, b + 32 + i] = 1.0
    return PT


def _build_program(nlayers, kp=8):
    """kp: number of live 128-key pieces for cross-attention (ceil(max
    input_length / 128)); keys >= kp*128 are masked for every sample so
    their K/V/scores are skipped entirely."""
    nc = bacc.Bacc("TRN2", num_devices=8)

    def din(name, shape, dtype=BF16):
        return nc.dram_tensor(name, shape, dtype, kind="ExternalInput")

    t = {}
    t["y0"] = din("y0", [C, TO], BF16)
    t["featT"] = din("featT", [C, S], F8E4)
    t["memmask"] = din("memmask", [S, 1], F32)
    t["mbig"] = din("mbig", [4, 128, TO], BF16)
    t["mtail"] = din("mtail", [2, TO], BF16)
    t["cosq"] = din("cosq", [128, TO], BF16)
    t["sinq"] = din("sinq", [128, TO], BF16)
    t["cosk"] = din("cosk", [128, TK], BF16)
    t["sink"] = din("sink", [128, TK], BF16)
    t["wtgt"] = din("wtgt", [C, TO])
    t["ptsw"] = din("ptsw", [128, 128])
    for nm in ["wqm", "wom", "wqt", "wkt", "wvt", "wot", "wkm", "wvm"]:
        t[nm] = din(nm, [nlayers, C, C], F8E5)
    t["w1"] = din("w1", [nlayers, C, FF], F8E5)
    t["w2"] = din("w2", [nlayers, FF, C], F8E5)
    t["wout"] = din("wout", [C, V], F8E5)

    t["out_se"] = nc.dram_tensor("out_se", [1, TO], F32, kind="ExternalOutput")
    t["out_tl"] = nc.dram_tensor("out_tl", [1, TO], F32, kind="ExternalOutput")

    t["ag_in"] = [nc.dram_tensor(f"agi{l}", [C, TO], F8E4, kind="Internal")
                  for l in range(nlayers)]
    t["ag_out"] = [nc.dram_tensor(f"ago{l}", [2, C, TO], F8E4, kind="Internal")
                   for l in range(nlayers)]
    t["RG"] = [[0, 1], [2, 3], [4, 5], [6, 7]]

    with tile.TileContext(nc) as tc:
        import contextlib
        with contextlib.ExitStack() as ctx:
            with nc.allow_low_precision(
                    reason="bf16 softmax denominators / LN stats are within "
                           "the 2e-2 output tolerance"):
                _build_body(nc, tc, nlayers, t, ctx, kp)
    nc.finalize()
    return nc


def _build_body(nc, tc, nlayers, t, ctx, kp):
    P = 128
    ec = ctx.enter_context
    persist = ec(tc.tile_pool(name="persist", bufs=1))
    wk = ec(tc.tile_pool(name="wk", bufs=2))     # [128,8,1024] bf16 weight mats
    w1p = ec(tc.tile_pool(name="w1p", bufs=3))   # [128,8,1024] bf16 ffn/wout
    pbf = ec(tc.tile_pool(name="pbf", bufs=4))   # [128,TO] bf16 exp'd scores
    scr = ec(tc.tile_pool(name="scr", bufs=3))   # [128,TO] fp32 scratch
    scrw = ec(tc.tile_pool(name="scrw", bufs=3))  # [128,512] scratch
    vrow = ec(tc.tile_pool(name="vrow", bufs=4))  # [1,TO] fp32 rows
    vrb = ec(tc.tile_pool(name="vrb", bufs=6))   # [1,TO] bf16 rows
    kvp = ec(tc.tile_pool(name="kvp", bufs=1))   # per-layer kv tiles
    hbf_p = ec(tc.tile_pool(name="hbf_p", bufs=8))  # ffn hidden tiles

    ps_a = ec(tc.tile_pool(name="ps_a", bufs=4, space="PSUM"))
    ps_b = ec(tc.tile_pool(name="ps_b", bufs=1, space="PSUM"))
    ps_o = ec(tc.tile_pool(name="ps_o", bufs=3, space="PSUM"))

    def pt3(nm, n, w, dtype):
        big = persist.tile([P, n, w], dtype, name=nm)
        return big, [big[:, i, :] for i in range(n)]

    yT3, yT = pt3("yT", 8, TO, BF16)
    xn3, xn = pt3("xn", 8, TO, BF16)
    xn83, xn8 = pt3("xn8", 8, TO, F8E4)
    # xh tiles padded to TOP-stride so DoubleRow can use them as the
    # stationary operand (pair-step must be a multiple of 16 bytes)
    xh03 = persist.tile([P, 8, TOP], F8E4, name="xh0")
    xh13 = persist.tile([P, 8, TOP], F8E4, name="xh1")
    ftT3, ftT = pt3("ftT", 8, S, F8E4)
    qm3, qm = pt3("qm", 8, TO, BF16)
    qt3, qt = pt3("qt", 8, TO, BF16)
    osb3, osb = pt3("osb", 8, TO, BF16)   # cross attn o
    osb83, osb8 = pt3("osb8", 8, TO, F8E4)
    osb283, osb28 = pt3("osb28", 8, TO, F8E4)
    osb23, osb2 = pt3("osb2", 8, TO, BF16)  # self attn o
    mmask3, mmask = pt3("mmask", 8, 1, F32)
    mbig3, mbig_t = pt3("mbigt", 4, TO, BF16)
    mtail_t = persist.tile([2, TO], BF16, name="mtailt")
    cq = persist.tile([P, TO], BF16, name="cq")
    sq = persist.tile([P, TO], BF16, name="sq")
    ck = persist.tile([P, TK], BF16, name="ck")
    sk = persist.tile([P, TK], BF16, name="sk")
    wtg3, wtg = pt3("wtg", 8, TO, BF16)
    ptw = persist.tile([P, P], BF16, name="ptw")
    ones_k = persist.tile([P, 1], BF16, name="ones_k")
    ones_kf = persist.tile([P, 1], F32, name="ones_kf")
    ones_r64 = persist.tile([1, 64], BF16, name="ones_r64")
    ones_r128 = persist.tile([1, P], BF16, name="ones_r128")
    epsr = persist.tile([1, 1], F32, name="epsr")

    dma = nc.sync.dma_start
    wdma = nc.gpsimd.dma_start

    def chunked(dr, p=P):
        return dr.rearrange("(k p) x -> p k x", p=p)

    dma(yT3[:], chunked(t["y0"]))
    dma(ftT3[:], chunked(t["featT"]))
    dma(mmask3[:], chunked(t["memmask"]))
    dma(wtg3[:], chunked(t["wtgt"]))
    dma(mbig3[:], t["mbig"].rearrange("c p t -> p c t"))
    dma(mtail_t[:], t["mtail"][:, :])
    dma(cq[:], t["cosq"][:, :])
    dma(sq[:], t["sinq"][:, :])
    dma(ck[:], t["cosk"][:, :])
    dma(sk[:], t["sink"][:, :])
    dma(ptw[:], t["ptsw"][:, :])
    nc.vector.memset(ones_k[:], 1.0)
    nc.vector.memset(ones_kf[:], 1.0)
    nc.vector.memset(ones_r64[:], 1.0)
    nc.vector.memset(ones_r128[:], 1.0)
    nc.vector.memset(epsr[:], 1e-5)

    def layernorm(src_tiles, out_tiles, tag):
        """out (bf16) = (src - mu)/sqrt(var+eps); src tiles are bf16.

        Stat matmuls read the bf16 residual directly; rsqrt is
        exp(-0.5*ln(var+eps)) so only the exp/ln ACT table set is needed.
        """
        sum1 = ps_o.tile([1, TO], F32, name="psb_o")
        sum2 = ps_o.tile([1, TO], F32, name="psb_o")
        # emit the squares first so the PE sum chain streams without waiting
        # per-k on ACT (engine queues execute in emission order)
        ysqs = []
        for k in range(8):
            ysq = pbf.tile([P, TO], BF16, name="lnb", bufs=12)
            nc.scalar.square(out=ysq[:], in_=src_tiles[k][:])
            ysqs.append(ysq)
        for k in range(8):
            nc.tensor.matmul(sum1[:], ones_k[:], src_tiles[k][:],
                             start=(k == 0), stop=(k == 7))
            nc.tensor.matmul(sum2[:], ones_k[:], ysqs[k][:],
                             start=(k == 0), stop=(k == 7))
        mub_r = vrb.tile([1, TO], BF16, name="vrb")
        nc.scalar.activation(out=mub_r[:], in_=sum1[:], func=AF.Copy,
                             scale=1.0 / C)
        mub = ps_a.tile([P, 512], F32, name="psa")[:, :TO]
        nc.tensor.matmul(mub, ones_r128[:], mub_r[:], start=True, stop=True)
        mu = vrow.tile([1, TO], F32, name="vrow")
        nc.scalar.mul(out=mu[:], in_=sum1[:], mul=1.0 / C)
        musq = vrow.tile([1, TO], F32, name="vrow")
        nc.vector.tensor_mul(out=musq[:], in0=mu[:], in1=mu[:])
        var = vrow.tile([1, TO], F32, name="vrow")
        nc.scalar.activation(out=var[:], in_=sum2[:], func=AF.Copy, scale=1.0 / C)
        nc.vector.tensor_sub(out=var[:], in0=var[:], in1=musq[:])
        lnv = vrow.tile([1, TO], F32, name="vrow")
        nc.scalar.activation(out=lnv[:], in_=var[:], func=AF.Ln, bias=epsr[:])
        rinv_b = vrb.tile([1, TO], BF16, name="vrb")
        nc.scalar.activation(out=rinv_b[:], in_=lnv[:], func=AF.Exp, scale=-0.5)
        rsb = ps_a.tile([P, 512], F32, name="psa")[:, :TO]
        nc.tensor.matmul(rsb, ones_r128[:], rinv_b[:], start=True, stop=True)
        mub_s = scrw.tile([P, TO], BF16, name="lnbc", bufs=2)
        nc.vector.tensor_copy(out=mub_s[:], in_=mub)
        rsb_s = scrw.tile([P, TO], BF16, name="lnbc", bufs=2)
        nc.vector.tensor_copy(out=rsb_s[:], in_=rsb)
        for k in range(8):
            d = pbf.tile([P, TO], BF16, name="lnd", bufs=2)
            nc.vector.tensor_sub(out=d[:], in0=src_tiles[k][:], in1=mub_s[:])
            nc.vector.tensor_tensor(out=out_tiles[k][:], in0=d[:], in1=rsb_s[:],
                                    op=ALU.mult)

    def load_w(dram, l, dt=BF16):
        # fp8 weight blocks ride in the w1p pool (same 16KB slot budget)
        wt = (wk.tile([P, 8, C], BF16, name="wk") if dt == BF16
              else w1p.tile([P, 8, C], dt, name="w1p"))
        wdma(wt[:], chunked(dram[l]))
        sl = [wt[:, k, :] for k in range(8)]
        sl.append(wt)
        return sl

    def proj_T(wtiles, rhs_tiles, consume):
        for m in range(8):
            pt = ps_a.tile([P, 512], F32, name="psa")[:, :TO]
            for k in range(8):
                nc.tensor.matmul(pt, wtiles[k][:, m * P:(m + 1) * P],
                                 rhs_tiles[k][:, :TO],
                                 start=(k == 0), stop=(k == 7))
            consume(pt, m)

    def rope1(pt, c0, c1, ctab):
        """Stage 1: pb = bf16(pt); a = pb*cos (all-bf16 DVE runs at 2x)."""
        w = c1 - c0
        pb = scrw.tile([P, TO], BF16, name="ropep")[:, :w]
        nc.vector.tensor_copy(out=pb, in_=pt)
        a = scrw.tile([P, TO], BF16, name="ropea")[:, :w]
        nc.vector.tensor_tensor(out=a, in0=pb, in1=ctab[:, c0:c1], op=ALU.mult)
        return a, pb

    def rope2(st, c0, c1, stab, out_bf):
        """Stage 2 (emit one chunk later): out = a + (PT.T@pb)*sin."""
        a, pb = st
        w = c1 - c0
        swp = ps_b.tile([P, TO], F32, name="psb")[:, :w]
        nc.tensor.matmul(swp, ptw[:], pb, start=True, stop=True)
        b = scrw.tile([P, TO], BF16, name="ropeb", bufs=2)[:, :w]
        nc.vector.tensor_tensor(out=b, in0=swp, in1=stab[:, c0:c1], op=ALU.mult)
        nc.vector.tensor_tensor(out=out_bf, in0=a, in1=b, op=ALU.add)

    def attention(q_tiles, k_tiles, v_tiles, pieces, o_tiles, o8_tiles,
                  mask_mode, tag, filler=None):
        """pieces: (kci, kr0, kr1, qc0, qc1) — key-slot rows x query cols.

        Piece 0 must span the full query range (its start=True write fills
        oaug's has_written bits for the whole accumulation region). Self
        masks are multiplicative {0,1} bf16 applied to exp(scores).

        `filler()` is invoked once per head to emit independent PE work into
        the queue — softmax (exp) throughput on the ACT engine is the
        bottleneck inside attention, so extra matmuls slot into PE gaps.
        """
        npc = len(pieces)

        rcs = {}

        def finalize(h, oaug):
            """Evacuate head h's oaug unnormalized (bf16) + its reciprocal
            softmax denominator; the normalizing multiply runs two heads
            later (norm) so the PE never waits on the DVE reciprocal."""
            ht, hb = h // 2, (h % 2) * 64
            nc.vector.tensor_copy(out=o_tiles[ht][hb:hb + 64, :],
                                  in_=oaug[0:64, :])
            rc = vrb.tile([1, TO], BF16, name="vrb")
            nc.vector.reciprocal(out=rc[:], in_=oaug[64:65, :])
            rcs[h] = rc

        def norm(m):
            rb = ps_a.tile([P, 512], F32, name="psa")[:, :TO]
            nc.tensor.matmul(rb[0:64, :], ones_r64[:], rcs.pop(2 * m)[:],
                             start=True, stop=True)
            nc.tensor.matmul(rb[64:128, :], ones_r64[:], rcs.pop(2 * m + 1)[:],
                             start=True, stop=True)
            nc.vector.tensor_tensor(out=o8_tiles[m][:], in0=o_tiles[m][:],
                                    in1=rb, op=ALU.mult)

        pending = []  # (h, oaug) of recent heads, not yet evacuated
        normq = []
        for h in range(H):
            ht, hb = h // 2, (h % 2) * 64
            oaug = ps_o.tile([65, TO], F32, name="psb_o")
            ps = []
            for j, piece in enumerate(pieces):
                (kci, kr0, kr1, qc0, qc1), mrng = piece[:5], piece[5:]
                rows = kr1 - kr0
                w = qc1 - qc0
                sp = ps_a.tile([P, 512], F32, name="psa")[:rows, :w]
                nc.tensor.matmul(
                    sp, k_tiles[ht][hb:hb + 64, kr0:kr1],
                    q_tiles[ht][hb:hb + 64, qc0:qc1],
                    start=True, stop=True, tile_position=(hb, 0),
                )
                p = pbf.tile([P, TO], BF16, name="pbf")[:rows, :w]
                if mask_mode == "mem":
                    nc.scalar.activation(out=p, in_=sp, func=AF.Exp,
                                         bias=mmask[kci][:rows, :])
                else:
                    nc.scalar.activation(out=p, in_=sp, func=AF.Exp)
                    mc0, mc1 = mrng
                    mt = (mbig_t[kci][:rows, mc0:mc1] if kci < 4
                          else mtail_t[0:rows, mc0:mc1])
                    pm = p[:, mc0 - qc0:mc1 - qc0]
                    nc.vector.tensor_tensor(out=pm, in0=pm, in1=mt,
                                            op=ALU.mult)
                ps.append((j, kci, rows, qc0, qc1, p))
            if normq:
                norm(normq.pop(0))
            if len(pending) >= 1:
                finalize(*pending.pop(0))
                if h - 1 >= 1 and (h - 1) % 2 == 1:
                    normq.append((h - 1) // 2)
            for (j, kci, rows, qc0, qc1, p) in ps:
                nc.tensor.matmul(oaug[:, qc0:qc1],
                                 v_tiles[kci][:rows, h * 65:(h + 1) * 65],
                                 p, start=(j == 0), stop=(j == npc - 1))
            pending.append((h, oaug))
            if filler is not None:
                filler()
        for pd in pending:
            finalize(*pd)
        for m in normq:
            norm(m)
        for m in range(8):
            if 2 * m in rcs:
                norm(m)

    def make_v(w3, src3d, src_sel, vtiles):
        """v tiles [rows, 16*65] bf16 with a ones column per head (fp8
        DoubleRow: both ftT and wv are fp8, contraction 256/pass)."""
        for i, (c0, c1) in enumerate(src_sel):
            rows = c1 - c0
            vt = vtiles[i]
            vt3 = vt.rearrange("r (h x) -> r h x", x=65)
            for half in range(2):
                pv = ps_a.tile([P, 512], F32, name="psa")[:rows, :]
                for c in range(4):
                    nc.tensor.matmul(
                        pv, src3d[:, 2 * c:2 * c + 2, c0:c1],
                        w3[:, 2 * c:2 * c + 2,
                           half * 512:(half + 1) * 512],
                        start=(c == 0), stop=(c == 3), perf_mode=DROW)
                src3 = pv.rearrange("r (h d) -> r h d", d=64)
                nc.vector.tensor_copy(
                    out=vt3[:rows, half * 8:(half + 1) * 8, 0:64], in_=src3)
            nc.vector.memset(vt3[:rows, :, 64:65], 1.0)

    # ================= layers =================
    for l in range(nlayers):
        layernorm(yT, xn8, f"ln1_{l}")
        dma(chunked(t["ag_in"][l]), xn83[:])
        nc.gpsimd.collective_compute(
            "AllGather", ALU.bypass, replica_groups=t["RG"],
            ins=[t["ag_in"][l][:]], outs=[t["ag_out"][l][:]],
        )
        dma(xh03[:, :, :TO], chunked(t["ag_out"][l][0]))
        dma(xh13[:, :, :TO], chunked(t["ag_out"][l][1]))

        # ---- cross attention (overlaps the AllGather) ----
        wq = load_w(t["wqm"], l, F8E5)[8]
        for m in range(8):
            pt = ps_a.tile([P, 512], F32, name="psa")[:, :TO]
            for c in range(4):
                nc.tensor.matmul(pt, wq[:, 2 * c:2 * c + 2, m * P:(m + 1) * P],
                                 xn83[:, 2 * c:2 * c + 2, :],
                                 start=(c == 0), stop=(c == 3), perf_mode=DROW)
            nc.vector.tensor_copy(out=qm[m][:], in_=pt)
        wv = load_w(t["wvm"], l, F8E5)[8]
        vm = [kvp.tile([P, 16 * 65], BF16, name=f"vm{i}") for i in range(kp)]
        make_v(wv, ftT3, [(i * P, (i + 1) * P) for i in range(kp)], vm)
        wkm_t = load_w(t["wkm"], l, F8E5)[8]
        km = [kvp.tile([P, S], BF16, name=f"km{i}") for i in range(8)]
        kcols = [(0, min(512, kp * P))]
        if kp * P > 512:
            kcols.append((512, kp * P))
        for (s0, s1) in kcols:
            for m in range(8):
                pt = ps_a.tile([P, 512], F32, name="psa")[:, :s1 - s0]
                for c in range(4):
                    nc.tensor.matmul(
                        pt, wkm_t[:, 2 * c:2 * c + 2, m * P:(m + 1) * P],
                        ftT3[:, 2 * c:2 * c + 2, s0:s1],
                        start=(c == 0), stop=(c == 3), perf_mode=DROW)
                nc.vector.tensor_copy(out=km[m][:, s0:s1], in_=pt)
        # ---- self-attn projections, emitted as filler units inside the
        # (ACT-bound) cross-attention head loop so the PE queue never drains.
        rpend = []

        def flush_rope(limit=2):
            while len(rpend) > limit:
                st, c0, c1, stab, dst = rpend.pop(0)
                rope2(st, c0, c1, stab, dst)

        wcur = {}
        kt = [kvp.tile([P, TK], BF16, name=f"kt{i}") for i in range(8)]
        vt_tiles = [kvp.tile([P, 16 * 65], BF16, name=f"vt{i}") for i in range(4)]
        vt_tiles.append(kvp.tile([2, 16 * 65], BF16, name="vt4"))
        units = []

        def u_load(key, dram):
            def f():
                wcur[key] = load_w(dram, l, F8E5)[8]
            return f

        def u_qt(m):
            def f():
                w3 = wcur["qt"]
                pt = ps_a.tile([P, 512], F32, name="psa")[:, :TO]
                for c in range(4):
                    nc.tensor.matmul(
                        pt, w3[:, 2 * c:2 * c + 2, m * P:(m + 1) * P],
                        xn83[:, 2 * c:2 * c + 2, :],
                        start=(c == 0), stop=(c == 3), perf_mode=DROW)
                rpend.append((rope1(pt, 0, TO, cq), 0, TO, sq, qt[m][:]))
                flush_rope()
            return f

        def u_kt(m, src3d, c0, cw):
            def f():
                w3 = wcur["kt"]
                pt = ps_a.tile([P, 512], F32, name="psa")[:, :cw]
                for c in range(4):
                    nc.tensor.matmul(
                        pt, w3[:, 2 * c:2 * c + 2, m * P:(m + 1) * P],
                        src3d[:, 2 * c:2 * c + 2, :cw],
                        start=(c == 0), stop=(c == 3), perf_mode=DROW)
                rpend.append((rope1(pt, c0, c0 + cw, ck), c0, c0 + cw, sk,
                              kt[m][:, c0:c0 + cw]))
                flush_rope()
            return f

        def u_vt(i, src3d, c0, c1, half):
            def f():
                rows = c1 - c0
                w3 = wcur["vt"]
                vt3 = vt_tiles[i].rearrange("r (h x) -> r h x", x=65)
                pv = ps_a.tile([P, 512], F32, name="psa")[:rows, :]
                # xh stationary: TOP-padded stride makes the pair-step
                # 16B-aligned, so DoubleRow (256-deep fp8) applies
                for c in range(4):
                    nc.tensor.matmul(
                        pv, src3d[:, 2 * c:2 * c + 2, c0:c1],
                        w3[:, 2 * c:2 * c + 2, half * 512:(half + 1) * 512],
                        start=(c == 0), stop=(c == 3), perf_mode=DROW)
                src3 = pv.rearrange("r (h d) -> r h d", d=64)
                nc.vector.tensor_copy(
                    out=vt3[:rows, half * 8:(half + 1) * 8, 0:64], in_=src3)
                if half == 1:
                    nc.vector.memset(vt3[:rows, :, 64:65], 1.0)
            return f

        units.append(u_load("qt", t["wqt"]))
        for m in range(8):
            units.append(u_qt(m))
        units.append(u_load("kt", t["wkt"]))
        for m in range(8):
            for (src3d, c0, cw) in [(xh03, 0, 256), (xh13, 256, TO)]:
                units.append(u_kt(m, src3d, c0, cw))
        units.append(u_load("vt", t["wvt"]))
        for i, (src3d, c0, c1) in enumerate(
                [(xh03, 0, 128), (xh03, 128, 256), (xh13, 0, 128),
                 (xh13, 128, 256), (xh13, 256, TO)]):
            for half in range(2):
                units.append(u_vt(i, src3d, c0, c1, half))
        units.append(lambda: flush_rope(0))

        def cross_filler(n=2):
            for _ in range(n):
                if units:
                    units.pop(0)()

        attention(qm, km, vm,
                  [(i, i * P, (i + 1) * P, 0, TO) for i in range(kp)],
                  osb, osb8, "mem", f"x{l}", filler=cross_filler)
        while units:
            units.pop(0)()

        # causal-trimmed pieces: key slots [r0A|r0B|r1A|r1B|tail] vs own
        # query cols; slot 0 must span the full query range (start=True).
        # mask multiplies cover only the (mc0, mc1) sub-ranges with zeros.
        attention(qt, kt, vt_tiles,
                  [(0, 0, 128, 0, TO, 0, 128), (1, 128, 256, 128, TO, 128, TO),
                   (2, 256, 384, 0, TO, 0, 128), (3, 384, 512, 128, TO, 128, TO),
                   (4, 512, 514, 256, TO, 256, TO)],
                  osb2, osb28, "self", f"s{l}")

        # ---- combined output projections (DVE: single add into y) ----
        wo1 = load_w(t["wom"], l, F8E5)[8]
        wo2 = load_w(t["wot"], l, F8E5)[8]
        for m in range(8):
            pt = ps_a.tile([P, 512], F32, name="psa")[:, :TO]
            for c in range(4):
                nc.tensor.matmul(
                    pt, wo1[:, 2 * c:2 * c + 2, m * P:(m + 1) * P],
                    osb83[:, 2 * c:2 * c + 2, :],
                    start=(c == 0), stop=False, perf_mode=DROW)
            for c in range(4):
                nc.tensor.matmul(
                    pt, wo2[:, 2 * c:2 * c + 2, m * P:(m + 1) * P],
                    osb283[:, 2 * c:2 * c + 2, :],
                    start=False, stop=(c == 3), perf_mode=DROW)
            nc.vector.tensor_tensor(out=yT[m][:], in0=pt, in1=yT[m][:],
                                    op=ALU.add)

        # ---- FFN (fp8 DoubleRow: h = gelu(xn8 @ W1sub), y += W2sub^T h) ----
        layernorm(yT, xn8, f"ln2_{l}")
        for sub in range(4):
            w1b = w1p.tile([P, 8, 1024], F8E5, name="w1p")
            wdma(w1b[:], chunked(t["w1"][l, :, sub * 1024:(sub + 1) * 1024]))
            hsub = []
            for mc in range(4):
                hp = hbf_p.tile([P, 2, TO], F8E4, name="hbf")
                for j in range(2):
                    m = 2 * mc + j
                    pt = ps_a.tile([P, 512], F32, name="psa")[:, :TO]
                    for c in range(4):
                        nc.tensor.matmul(
                            pt, w1b[:, 2 * c:2 * c + 2, m * P:(m + 1) * P],
                            xn83[:, 2 * c:2 * c + 2, :],
                            start=(c == 0), stop=(c == 3), perf_mode=DROW)
                    nc.scalar.activation(out=hp[:, j, :], in_=pt, func=AF.Gelu)
                hsub.append(hp)
            w2b = w1p.tile([P, 8, 1024], F8E5, name="w1p")
            wdma(w2b[:], chunked(t["w2"][l, sub * 1024:(sub + 1) * 1024, :]))
            for m in range(8):
                pt = ps_a.tile([P, 512], F32, name="psa")[:, :TO]
                for c in range(4):
                    nc.tensor.matmul(
                        pt, w2b[:, 2 * c:2 * c + 2, m * P:(m + 1) * P],
                        hsub[c][:, :, :],
                        start=(c == 0), stop=(c == 3), perf_mode=DROW)
                nc.vector.tensor_tensor(out=yT[m][:], in0=pt, in1=yT[m][:],
                                        op=ALU.add)

    # ================= head =================
    # logits computed [128-vocab-tile, TO] (vocab on partitions): wout is
    # the DoubleRow stationary, xn8 the moving operand; exp'd tiles are
    # reduced over vocab partitions by an accumulating ones-matmul.
    layernorm(yT, xn, "lnf")
    for k in range(8):
        nc.scalar.activation(out=xn8[k][:], in_=xn[k][:], func=AF.Copy)
    sep = ps_o.tile([1, TO], F32, name="psb_o")
    for nvp in range(8):
        wb = w1p.tile([P, 8, 1024], F8E5, name="w1p")
        wdma(wb[:], chunked(t["wout"][:, nvp * 1024:(nvp + 1) * 1024]))
        for vsub in range(8):
            vi = nvp * 8 + vsub
            pt = ps_a.tile([P, 512], F32, name="psa")[:, :TO]
            for c in range(4):
                nc.tensor.matmul(
                    pt, wb[:, 2 * c:2 * c + 2, vsub * P:(vsub + 1) * P],
                    xn83[:, 2 * c:2 * c + 2, :],
                    start=(c == 0), stop=(c == 3), perf_mode=DROW)
            esc = pbf.tile([P, TO], BF16, name="pbf")
            nc.scalar.activation(out=esc[:], in_=pt, func=AF.Exp)
            nc.tensor.matmul(sep[:], ones_k[:], esc[:],
                             start=(vi == 0), stop=(vi == 63))
    ses = vrow.tile([1, TO], F32, name="vrow")
    nc.scalar.activation(out=ses[:], in_=sep[:], func=AF.Copy)
    dma(t["out_se"][:, :], ses[:])

    # target logit: tl = sum_c wtgt * yf (bf16-rounded, matching logits path)
    tlacc = scr.tile([P, TO], F32, name="tlacc", bufs=1)
    for k in range(8):
        xf = scr.tile([P, TO], F32, name="scr")
        nc.vector.tensor_tensor(out=xf[:], in0=xn[k][:], in1=wtg[k][:],
                                op=ALU.mult)
        if k == 0:
            nc.vector.tensor_copy(out=tlacc[:], in_=xf[:])
        else:
            nc.vector.tensor_add(out=tlacc[:], in0=tlacc[:], in1=xf[:])
    tlp = ps_o.tile([1, TO], F32, name="psb_o")
    nc.tensor.matmul(tlp[:], ones_kf[:], tlacc[:], start=True, stop=True)
    tls = vrow.tile([1, TO], F32, name="vrow")
    nc.scalar.activation(out=tls[:], in_=tlp[:], func=AF.Copy)
    dma(t["out_tl"][:, :], tls[:])


# ======================= host side =======================

def _host_inputs(features, targets, input_lengths, target_lengths, wte, ln1_w,
                 Wq_m, Wk_m, Wv_m, Wo_m, Wq_t, Wk_t, Wv_t, Wo_t, ln2_w, W1,
                 W2, lnf_w, Wout, nlayers):
    bf = ml_dtypes.bfloat16
    f32 = np.float32
    features = np.asarray(features, f32)
    targets = np.asarray(targets).astype(np.int64)
    input_lengths = np.asarray(input_lengths).astype(np.int64)
    target_lengths = np.asarray(target_lengths).astype(np.int64)
    ln1_w = np.asarray(ln1_w, f32)
    ln2_w = np.asarray(ln2_w, f32)
    lnf_w = np.asarray(lnf_w, f32)

    n = targets.shape[0]
    prompt = np.concatenate(
        [np.full((n, 1), STX, np.int64), targets], axis=1)  # [N, T]
    tgt = np.concatenate([targets, np.zeros((n, 1), np.int64)], axis=1)
    tgt[np.arange(n), target_lengths] = ETX

    perm = _rope_perm()
    pos_k = _key_global_idx()
    f8 = 1.0 / np.sqrt(np.float32(D))

    def cast(x):
        return np.ascontiguousarray(np.asarray(x, f32)).astype(bf)

    e4 = ml_dtypes.float8_e4m3
    e5 = ml_dtypes.float8_e5m2

    def cast8(x):
        return np.ascontiguousarray(np.asarray(x, f32)).astype(e5)

    sharedw = {
        "wqm": cast8(np.asarray(Wq_m, f32)[:nlayers] * ln1_w[:nlayers, :, None] * f8),
        "wkm": cast8(np.asarray(Wk_m, f32)[:nlayers]),
        "wvm": cast8(np.asarray(Wv_m, f32)[:nlayers]),
        "wom": cast8(np.asarray(Wo_m, f32)[:nlayers]),
        "wqt": cast8((np.asarray(Wq_t, f32)[:nlayers] * ln1_w[:nlayers, :, None]
                      * f8)[:, :, perm]),
        "wkt": cast8((np.asarray(Wk_t, f32)[:nlayers]
                      * ln1_w[:nlayers, :, None])[:, :, perm]),
        "wvt": cast8(np.asarray(Wv_t, f32)[:nlayers] * ln1_w[:nlayers, :, None]),
        "wot": cast8(np.asarray(Wo_t, f32)[:nlayers]),
        "w1": cast8(np.asarray(W1, f32)[:nlayers] * ln2_w[:nlayers, :, None]),
        "w2": cast8(np.asarray(W2, f32)[:nlayers]),
        "ptsw": _pswap().astype(bf),
    }
    wout_f = np.asarray(Wout, f32) * lnf_w[:, None]
    wout_bf = (np.asarray(Wout, f32) * lnf_w[:, None]).astype(e5)
    sharedw["wout"] = wout_bf

    y0_all = np.asarray(wte, f32)[prompt]  # [N, T, C]

    in_maps, meta = [], []
    for core in range(8):
        nb, r = core // 2, core % 2
        own = _own_global_idx(r)
        ownpos = np.maximum(own, 0)
        y0T = np.where(own[None, :] >= 0, y0_all[nb][ownpos].T, 0.0).astype(bf)
        cosq_, sinq_ = _rope_tables(own)
        cosk_, sink_ = _rope_tables(pos_k)
        mbig, mtail = _self_masks(r)
        memmask = np.where(np.arange(S) < input_lengths[nb], 0.0,
                           NEG).astype(f32)[:, None]
        padmask = (own >= 0).astype(f32)[None, :]
        wtgt = np.where(own[None, :] >= 0,
                        wout_bf.astype(f32)[:, tgt[nb][ownpos]], 0.0).astype(bf)
        im = {
            "y0": y0T,
            "featT": np.ascontiguousarray(features[nb].T).astype(e4),
            "memmask": memmask,
            "mbig": mbig.astype(bf), "mtail": mtail.astype(bf),
            "padmask": padmask,
            "cosq": cosq_.astype(bf), "sinq": sinq_.astype(bf),
            "cosk": cosk_.astype(bf), "sink": sink_.astype(bf),
            "wtgt": wtgt,
        }
        im.update(sharedw)
        in_maps.append(im)
        valid = np.where(own >= 0, (tgt[nb][ownpos] != 0), False)
        meta.append((nb, own, valid))
    return in_maps, meta


def kernel(features, targets, input_lengths, target_lengths, wte, ln1_w,
           Wq_m, Wk_m, Wv_m, Wo_m, Wq_t, Wk_t, Wv_t, Wo_t, ln2_w, W1, W2,
           lnf_w, Wout):
    nlayers = L
    # specialize the program to the live cross-attn key range: keys beyond
    # max(input_lengths) rounded up to 128 are masked for every sample
    kp = int(min(8, max(1, -(-int(np.max(input_lengths)) // 128))))
    key = (nlayers, kp)
    if key not in _prog_cache:
        _prog_cache[key] = _build_program(nlayers, kp)
    nc = _prog_cache[key]

    in_maps, meta = _host_inputs(
        features, targets, input_lengths, target_lengths, wte, ln1_w,
        Wq_m, Wk_m, Wv_m, Wo_m, Wq_t, Wk_t, Wv_t, Wo_t, ln2_w, W1, W2,
        lnf_w, Wout, nlayers)

    res = run_bass_kernel_spmd(nc, in_maps, core_ids=list(range(8)))
    globals()["LAST_RESULTS"] = res

    num, den = 0.0, 0.0
    for core in range(8):
        r = res.results[core]
        _, own, valid = meta[core]
        sumexp = r["out_se"][0]
        tl = r["out_tl"][0]
        nll = np.log(np.maximum(sumexp, 1e-300)) - tl
        num += float(np.sum(nll[valid]))
        den += float(np.sum(valid))
    return np.float32(num / max(den, 1.0))



# revision 33
# speedup vs baseline: 1.1710x; 1.0156x over previous
"""Trainium2 Bass kernel for nn_CTCAttentionDecoder.

12-layer transformer decoder (cross-attn over encoder memory + causal
self-attn with rotary embeddings + FFN) -> LM head -> masked NLL loss.

Parallelization: 8 NeuronCores = 4 pairs (one batch sample each); within a
pair, decoder tokens are split between the two cores (interleaved 128-token
chunks to balance causal attention work). K/V are computed redundantly on
both cores so the only per-layer communication is a single pair-AllGather of
the layer-normed activations (fp8). The LM head runs with full vocab on
each core for its own tokens; the host combines 8 per-token partial results
into the scalar loss.

Precision: residual stream and softmax/layernorm statistics in fp32;
attention scores/AV in bf16; all weight projections (QKVO, FFN, K/V-mem,
LM head) run in fp8 (e5m2 weights x e4m3 activations) with fp32 PSUM
accumulation, using DoubleRow 256-deep contraction where the stationary
operand's pair-step is 16B-aligned. Final rel err vs the fp32 reference is
~1e-3, against a 2e-2 tolerance.

Scheduling: engine queues execute in emission order, so the build software-
pipelines everything: self-attn projections are emitted as filler units
inside the ACT-bound cross-attention head loop, rope's swap-matmul and the
softmax normalization run 1-2 chunks/heads behind their producers, and
causally-dead score/exp/mask work is skipped via per-piece query ranges.
"""

import os

import numpy as np
import ml_dtypes

import concourse.bacc as bacc
import concourse.mybir as mybir
import concourse.tile as tile
from concourse.bass_utils import run_bass_kernel_spmd

F32 = mybir.dt.float32
BF16 = mybir.dt.bfloat16
F8E4 = mybir.dt.float8e4
F8E5 = mybir.dt.float8e5
DROW = mybir.MatmulPerfMode.DoubleRow
AF = mybir.ActivationFunctionType
ALU = mybir.AluOpType

N, S, T0, C, H, D, NLAYERS, V, FF = 4, 1024, 512, 1024, 16, 64, 12, 8192, 4096
T = T0 + 1  # 513
TO = 258  # own tokens per core (incl. pad columns)
TOP = 272  # padded xh stride (16B-aligned pair-step for DoubleRow)
TK = 514  # key slots: [c0 | c3 | c1 | c2 | t512 | pad]
STX, ETX = 3, 4
NEG = -1e30

L = int(os.environ.get("K_LAYERS", str(NLAYERS)))

_prog_cache = {}


def _own_global_idx(r):
    """Global token index per own column; -1 for pad columns."""
    if r == 0:
        return np.concatenate([np.arange(0, 128), np.arange(384, 512), [-1, -1]])
    return np.concatenate([np.arange(128, 256), np.arange(256, 384), [512, -1]])


def _key_global_idx():
    """Global token index per key slot; -1 for the pad slot."""
    return np.concatenate(
        [np.arange(0, 128), np.arange(384, 512), np.arange(128, 256),
         np.arange(256, 384), [512, -1]]
    )


def _rope_tables(pos, rows=128):
    """cos/sin tables [rows, len(pos)]; row i uses theta_(i%32)."""
    th = (10000.0 ** (-2.0 * np.arange(32) / D))  # [32]
    ang = th[:, None] * np.maximum(pos, 0)[None, :].astype(np.float64)
    cos = np.cos(ang).astype(np.float32)
    sin = np.sin(ang).astype(np.float32)
    reps = rows // 32
    return np.tile(cos, (reps, 1)), np.tile(sin, (reps, 1))


def _self_masks(r):
    """Causal masks, multiplicative {0, 1}: [4, 128, TO] big chunks + [2, TO]
    tail. Applied to exp(scores) (p = exp(s) * m), so no NEG bias needed."""
    own = _own_global_idx(r)  # [TO]
    key = _key_global_idx()  # [TK]
    big = np.zeros((4, 128, TO), np.float32)
    for kc in range(4):
        kg = key[kc * 128:(kc + 1) * 128]
        big[kc] = (kg[:, None] <= own[None, :]).astype(np.float32)
    tail = (key[512:514, None] <= own[None, :]).astype(np.float32)
    tail[key[512:514] < 0, :] = 0.0  # pad key slot: never attended
    # Pad query columns: allow everything so rowsum > 0 (their output is
    # garbage-but-finite and never read; a fully-masked row gives 0/0 NaN
    # that would pollute real tokens through later layers).
    pad_q = own < 0
    big[:, :, pad_q] = 1.0
    tail[:, pad_q] = 1.0
    return big, tail


def _rope_perm():
    """Column permutation de-interleaving rotary pairs within each head."""
    p = np.arange(C).reshape(H, D)
    newd = np.concatenate([np.arange(0, D, 2), np.arange(1, D, 2)])
    return p[:, newd].reshape(-1)


def _pswap():
    """PT [128,128] with qswap = PT.T @ q: out[r]=-q[r+32], out[r+32]=q[r]."""
    PT = np.zeros((128, 128), np.float32)
    for b in range(0, 128, 64):
        for i in range(32):
            PT[b + 32 + i, b + i] = -1.0
            PT[b + i, b + 32 + i] = 1.0
    return PT


def _build_program(nlayers, kp=8, kclean=0):
    """kp: number of live 128-key pieces for cross-attention (ceil(max
    input_length / 128)); keys >= kp*128 are masked for every sample so
    their K/V/scores are skipped entirely. kclean: pieces < kclean are
    fully inside every sample's input_length, so their exp needs no mask
    bias (enables paired two-bank exps)."""
    nc = bacc.Bacc("TRN2", num_devices=8)

    def din(name, shape, dtype=BF16):
        return nc.dram_tensor(name, shape, dtype, kind="ExternalInput")

    t = {}
    t["y0"] = din("y0", [C, TO], BF16)
    t["featT"] = din("featT", [C, S], F8E4)
    t["memmask"] = din("memmask", [S, 1], F32)
    t["mm01"] = din("mm01", [S, 1], F32)
    t["mbig"] = din("mbig", [4, 128, TO], BF16)
    t["mtail"] = din("mtail", [2, TO], BF16)
    t["cosq"] = din("cosq", [128, TO], BF16)
    t["sinq"] = din("sinq", [128, TO], BF16)
    t["cosk"] = din("cosk", [128, TK], BF16)
    t["sink"] = din("sink", [128, TK], BF16)
    t["wtgt"] = din("wtgt", [C, TO])
    t["ptsw"] = din("ptsw", [128, 128])
    for nm in ["wqm", "wom", "wqt", "wkt", "wvt", "wot", "wkm", "wvm"]:
        t[nm] = din(nm, [nlayers, C, C], F8E5)
    t["w1"] = din("w1", [nlayers, C, FF], F8E5)
    t["w2"] = din("w2", [nlayers, FF, C], F8E5)
    t["wout"] = din("wout", [C, V], F8E5)

    t["out_se"] = nc.dram_tensor("out_se", [1, TO], F32, kind="ExternalOutput")
    t["out_tl"] = nc.dram_tensor("out_tl", [1, TO], F32, kind="ExternalOutput")

    t["ag_in"] = [nc.dram_tensor(f"agi{l}", [C, TO], F8E4, kind="Internal")
                  for l in range(nlayers)]
    t["ag_out"] = [nc.dram_tensor(f"ago{l}", [2, C, TO], F8E4, kind="Internal")
                   for l in range(nlayers)]
    t["RG"] = [[0, 1], [2, 3], [4, 5], [6, 7]]

    with tile.TileContext(nc) as tc:
        import contextlib
        with contextlib.ExitStack() as ctx:
            with nc.allow_low_precision(
                    reason="bf16 softmax denominators / LN stats are within "
                           "the 2e-2 output tolerance"):
                _build_body(nc, tc, nlayers, t, ctx, kp, kclean)
    nc.finalize()
    return nc


def _build_body(nc, tc, nlayers, t, ctx, kp, kclean):
    P = 128
    ec = ctx.enter_context
    persist = ec(tc.tile_pool(name="persist", bufs=1))
    wk = ec(tc.tile_pool(name="wk", bufs=2))     # [128,8,1024] bf16 weight mats
    w1p = ec(tc.tile_pool(name="w1p", bufs=6))   # [128,8,1024] bf16 ffn/wout
    pbf = ec(tc.tile_pool(name="pbf", bufs=4))   # [128,TO] bf16 exp'd scores
    scr = ec(tc.tile_pool(name="scr", bufs=3))   # [128,TO] fp32 scratch
    scrw = ec(tc.tile_pool(name="scrw", bufs=3))  # [128,512] scratch
    vrow = ec(tc.tile_pool(name="vrow", bufs=4))  # [1,TO] fp32 rows
    vrb = ec(tc.tile_pool(name="vrb", bufs=6))   # [1,TO] bf16 rows
    kvp = ec(tc.tile_pool(name="kvp", bufs=1))   # per-layer kv tiles
    hbf_p = ec(tc.tile_pool(name="hbf_p", bufs=8))  # ffn hidden tiles

    # 8 PSUM banks: ps_p = two 2-bank "pair" tiles (scores/gelu/head exp in
    # [128,2,512] so one ACT instruction covers two pieces), ps_a = two
    # 1-bank projection tiles, ps_o = two 1-bank accumulators (oaug / LN
    # sums; at most 2 live by construction).
    ps_p = ec(tc.tile_pool(name="ps_p", bufs=2, space="PSUM"))
    ps_a = ec(tc.tile_pool(name="ps_a", bufs=2, space="PSUM"))
    ps_o = ec(tc.tile_pool(name="ps_o", bufs=2, space="PSUM"))

    def pt3(nm, n, w, dtype):
        big = persist.tile([P, n, w], dtype, name=nm)
        return big, [big[:, i, :] for i in range(n)]

    yT3, yT = pt3("yT", 8, TO, BF16)
    xn3, xn = pt3("xn", 8, TO, BF16)
    xn83, xn8 = pt3("xn8", 8, TO, F8E4)
    # xh tiles padded to TOP-stride so DoubleRow can use them as the
    # stationary operand (pair-step must be a multiple of 16 bytes)
    xh03 = persist.tile([P, 8, TOP], F8E4, name="xh0")
    xh13 = persist.tile([P, 8, TOP], F8E4, name="xh1")
    ftT3, ftT = pt3("ftT", 8, S, F8E4)
    qm3, qm = pt3("qm", 8, TO, BF16)
    qt3, qt = pt3("qt", 8, TO, BF16)
    osb3, osb = pt3("osb", 8, TO, BF16)   # cross attn o
    osb83, osb8 = pt3("osb8", 8, TO, F8E4)
    osb283, osb28 = pt3("osb28", 8, TO, F8E4)
    osb23, osb2 = pt3("osb2", 8, TO, BF16)  # self attn o
    mmask3, mmask = pt3("mmask", 8, 1, F32)
    mm013, mm01t = pt3("mm01", 8, 1, F32)
    mbig3, mbig_t = pt3("mbigt", 4, TO, BF16)
    mtail_t = persist.tile([2, TO], BF16, name="mtailt")
    cq = persist.tile([P, TO], BF16, name="cq")
    sq = persist.tile([P, TO], BF16, name="sq")
    ck = persist.tile([P, TK], BF16, name="ck")
    sk = persist.tile([P, TK], BF16, name="sk")
    wtg3, wtg = pt3("wtg", 8, TO, BF16)
    ptw = persist.tile([P, P], BF16, name="ptw")
    ones_k = persist.tile([P, 1], BF16, name="ones_k")
    ones_kf = persist.tile([P, 1], F32, name="ones_kf")
    ones_r64 = persist.tile([1, 64], BF16, name="ones_r64")
    ones_r128 = persist.tile([1, P], BF16, name="ones_r128")
    epsr = persist.tile([1, 1], F32, name="epsr")

    dma = nc.sync.dma_start
    wdma = nc.gpsimd.dma_start

    def chunked(dr, p=P):
        return dr.rearrange("(k p) x -> p k x", p=p)

    dma(yT3[:], chunked(t["y0"]))
    dma(ftT3[:], chunked(t["featT"]))
    dma(mmask3[:], chunked(t["memmask"]))
    dma(mm013[:], chunked(t["mm01"]))
    dma(wtg3[:], chunked(t["wtgt"]))
    dma(mbig3[:], t["mbig"].rearrange("c p t -> p c t"))
    dma(mtail_t[:], t["mtail"][:, :])
    dma(cq[:], t["cosq"][:, :])
    dma(sq[:], t["sinq"][:, :])
    dma(ck[:], t["cosk"][:, :])
    dma(sk[:], t["sink"][:, :])
    dma(ptw[:], t["ptsw"][:, :])
    nc.vector.memset(ones_k[:], 1.0)
    nc.vector.memset(ones_kf[:], 1.0)
    nc.vector.memset(ones_r64[:], 1.0)
    nc.vector.memset(ones_r128[:], 1.0)
    nc.vector.memset(epsr[:], 1e-5)

    def layernorm(src_tiles, out_tiles, tag):
        """out (bf16) = (src - mu)/sqrt(var+eps); src tiles are bf16.

        Stat matmuls read the bf16 residual directly; rsqrt is
        exp(-0.5*ln(var+eps)) so only the exp/ln ACT table set is needed.
        """
        sum1 = ps_o.tile([1, TO], F32, name="psb_o")
        sum2 = ps_o.tile([1, TO], F32, name="psb_o")
        # emit the squares first so the PE sum chain streams without waiting
        # per-k on ACT (engine queues execute in emission order)
        ysqs = []
        for k in range(8):
            ysq = pbf.tile([P, TO], BF16, name="lnb", bufs=12)
            nc.scalar.square(out=ysq[:], in_=src_tiles[k][:])
            ysqs.append(ysq)
        for k in range(8):
            nc.tensor.matmul(sum1[:], ones_k[:], src_tiles[k][:],
                             start=(k == 0), stop=(k == 7))
            nc.tensor.matmul(sum2[:], ones_k[:], ysqs[k][:],
                             start=(k == 0), stop=(k == 7))
        mub_r = vrb.tile([1, TO], BF16, name="vrb")
        nc.scalar.activation(out=mub_r[:], in_=sum1[:], func=AF.Copy,
                             scale=1.0 / C)
        mub = ps_a.tile([P, 512], F32, name="psa")[:, :TO]
        nc.tensor.matmul(mub, ones_r128[:], mub_r[:], start=True, stop=True)
        mu = vrow.tile([1, TO], F32, name="vrow")
        nc.scalar.mul(out=mu[:], in_=sum1[:], mul=1.0 / C)
        musq = vrow.tile([1, TO], F32, name="vrow")
        nc.vector.tensor_mul(out=musq[:], in0=mu[:], in1=mu[:])
        var = vrow.tile([1, TO], F32, name="vrow")
        nc.scalar.activation(out=var[:], in_=sum2[:], func=AF.Copy, scale=1.0 / C)
        nc.vector.tensor_sub(out=var[:], in0=var[:], in1=musq[:])
        lnv = vrow.tile([1, TO], F32, name="vrow")
        nc.scalar.activation(out=lnv[:], in_=var[:], func=AF.Ln, bias=epsr[:])
        rinv_b = vrb.tile([1, TO], BF16, name="vrb")
        nc.scalar.activation(out=rinv_b[:], in_=lnv[:], func=AF.Exp, scale=-0.5)
        rsb = ps_a.tile([P, 512], F32, name="psa")[:, :TO]
        nc.tensor.matmul(rsb, ones_r128[:], rinv_b[:], start=True, stop=True)
        mub_s = scrw.tile([P, TO], BF16, name="lnbc", bufs=2)
        nc.vector.tensor_copy(out=mub_s[:], in_=mub)
        rsb_s = scrw.tile([P, TO], BF16, name="lnbc", bufs=2)
        nc.vector.tensor_copy(out=rsb_s[:], in_=rsb)
        for k in range(8):
            d = pbf.tile([P, TO], BF16, name="lnd", bufs=2)
            nc.vector.tensor_sub(out=d[:], in0=src_tiles[k][:], in1=mub_s[:])
            nc.vector.tensor_tensor(out=out_tiles[k][:], in0=d[:], in1=rsb_s[:],
                                    op=ALU.mult)

    def load_w(dram, l, dt=BF16):
        # fp8 weight blocks ride in the w1p pool (same 16KB slot budget)
        wt = (wk.tile([P, 8, C], BF16, name="wk") if dt == BF16
              else w1p.tile([P, 8, C], dt, name="w1p"))
        wdma(wt[:], chunked(dram[l]))
        sl = [wt[:, k, :] for k in range(8)]
        sl.append(wt)
        return sl

    def proj_T(wtiles, rhs_tiles, consume):
        for m in range(8):
            pt = ps_a.tile([P, 512], F32, name="psa")[:, :TO]
            for k in range(8):
                nc.tensor.matmul(pt, wtiles[k][:, m * P:(m + 1) * P],
                                 rhs_tiles[k][:, :TO],
                                 start=(k == 0), stop=(k == 7))
            consume(pt, m)

    def rope1(pt, c0, c1, ctab):
        """Stage 1: pb = bf16(pt) on ACT; a = pb*cos on Pool (both engines
        have slack; DVE is the critical engine)."""
        w = c1 - c0
        pb = scrw.tile([P, TO], BF16, name="ropep")[:, :w]
        nc.vector.tensor_copy(out=pb, in_=pt)
        a = scrw.tile([P, TO], BF16, name="ropea")[:, :w]
        nc.vector.tensor_tensor(out=a, in0=pb, in1=ctab[:, c0:c1], op=ALU.mult)
        return a, pb

    def rope2(st, c0, c1, stab, out_bf):
        """Stage 2 (emit one chunk later): out = a + (PT.T@pb)*sin."""
        a, pb = st
        w = c1 - c0
        swp = ps_a.tile([P, 512], F32, name="psa")[:, :w]
        nc.tensor.matmul(swp, ptw[:], pb, start=True, stop=True)
        b = scrw.tile([P, TO], BF16, name="ropeb", bufs=2)[:, :w]
        nc.vector.tensor_tensor(out=b, in0=swp, in1=stab[:, c0:c1], op=ALU.mult)
        nc.vector.tensor_tensor(out=out_bf, in0=a, in1=b, op=ALU.add)

    def attention(q_tiles, k_tiles, v_tiles, pieces, o_tiles, o8_tiles,
                  mask_mode, tag, filler=None, clean=0, groups=None):
        """pieces: (kci, kr0, kr1, qc0, qc1) — key-slot rows x query cols.

        Piece 0 must span the full query range (its start=True write fills
        oaug's has_written bits for the whole accumulation region). Self
        masks are multiplicative {0,1} bf16 applied to exp(scores).

        Scores land in 2-bank ps_p pair tiles; a same-width group whose
        pieces need no exp-bias (mem pieces < `clean`, or any self pieces)
        is exp'd by ONE ACT instruction over a [128,2,w] AP — the per-
        instruction ACT bubble is ~40% of a 258-wide exp, so pairing is a
        big ACT saving. `groups` orders score/exp work; AV accumulation
        keeps the original piece order.

        `filler()` is invoked once per head to emit independent PE work into
        the queue so the PE never drains while ACT chews on softmax.
        """
        npc = len(pieces)
        if groups is None:
            groups = [tuple(range(i, min(i + 2, npc)))
                      for i in range(0, npc, 2)]

        rcs = {}

        def finalize(h, oaug):
            """Evacuate head h's oaug unnormalized (bf16) + its reciprocal
            softmax denominator; the normalizing multiply runs two heads
            later (norm) so the PE never waits on the DVE reciprocal.
            Copies alternate DVE/ACT to balance the two engines."""
            ht, hb = h // 2, (h % 2) * 64
            nc.vector.tensor_copy(out=o_tiles[ht][hb:hb + 64, :],
                                  in_=oaug[0:64, :])
            rc = vrb.tile([1, TO], BF16, name="vrb")
            nc.vector.reciprocal(out=rc[:], in_=oaug[64:65, :])
            rcs[h] = rc

        def norm(m):
            rb = ps_a.tile([P, 512], F32, name="psa")[:, :TO]
            nc.tensor.matmul(rb[0:64, :], ones_r64[:], rcs.pop(2 * m)[:],
                             start=True, stop=True)
            nc.tensor.matmul(rb[64:128, :], ones_r64[:], rcs.pop(2 * m + 1)[:],
                             start=True, stop=True)
            nc.vector.tensor_tensor(out=o8_tiles[m][:], in0=o_tiles[m][:],
                                    in1=rb, op=ALU.mult)

        pending = []  # (h, oaug) of recent heads, not yet evacuated
        normq = []
        for h in range(H):
            ht, hb = h // 2, (h % 2) * 64
            oaug = ps_o.tile([65, TO], F32, name="psb_o")
            ps = [None] * npc
            for g in groups:
                w0 = pieces[g[0]][4] - pieces[g[0]][3]
                joint = (len(g) == 2
                         and pieces[g[1]][4] - pieces[g[1]][3] == w0)
                sp2 = ps_p.tile([P, 2, 512], F32, name="psp")
                for idx, j in enumerate(g):
                    (kci, kr0, kr1, qc0, qc1) = pieces[j][:5]
                    rows = kr1 - kr0
                    w = qc1 - qc0
                    nc.tensor.matmul(
                        sp2[:rows, idx, :w], k_tiles[ht][hb:hb + 64, kr0:kr1],
                        q_tiles[ht][hb:hb + 64, qc0:qc1],
                        start=True, stop=True, tile_position=(hb, 0),
                    )
                p2 = None
                if joint:
                    p2 = pbf.tile([P, 2, TO], BF16, name="pbf2", bufs=4)
                    nc.scalar.activation(out=p2[:, :, :w0],
                                         in_=sp2[:, :, :w0], func=AF.Exp)
                for idx, j in enumerate(g):
                    (kci, kr0, kr1, qc0, qc1), mrng = pieces[j][:5], pieces[j][5:]
                    rows = kr1 - kr0
                    w = qc1 - qc0
                    if joint:
                        p = p2[:rows, idx, :w]
                        if mask_mode == "mem" and j >= clean:
                            nc.vector.tensor_scalar_mul(
                                out=p, in0=p, scalar1=mm01t[kci][:rows, :])
                    else:
                        p = pbf.tile([P, TO], BF16, name="pbf")[:rows, :w]
                        if mask_mode == "mem":
                            nc.scalar.activation(out=p, in_=sp2[:rows, idx, :w],
                                                 func=AF.Exp,
                                                 bias=mmask[kci][:rows, :])
                        else:
                            nc.scalar.activation(out=p, in_=sp2[:rows, idx, :w],
                                                 func=AF.Exp)
                    if mask_mode != "mem" and mrng:
                        mc0, mc1 = mrng
                        mt = (mbig_t[kci][:rows, mc0:mc1] if kci < 4
                              else mtail_t[0:rows, mc0:mc1])
                        pm = p[:, mc0 - qc0:mc1 - qc0]
                        nc.vector.tensor_tensor(out=pm, in0=pm, in1=mt,
                                                op=ALU.mult)
                    ps[j] = (j, kci, rows, qc0, qc1, p)
            if normq:
                norm(normq.pop(0))
            if len(pending) >= 1:
                finalize(*pending.pop(0))
                if h - 1 >= 1 and (h - 1) % 2 == 1:
                    normq.append((h - 1) // 2)
            for (j, kci, rows, qc0, qc1, p) in ps:
                nc.tensor.matmul(oaug[:, qc0:qc1],
                                 v_tiles[kci][:rows, h * 65:(h + 1) * 65],
                                 p, start=(j == 0), stop=(j == npc - 1))
            pending.append((h, oaug))
            if filler is not None:
                filler()
        for pd in pending:
            finalize(*pd)
        for m in normq:
            norm(m)
        for m in range(8):
            if 2 * m in rcs:
                norm(m)

    def make_v(w3, src3d, src_sel, vtiles):
        """v tiles [rows, 16*65] bf16 with a ones column per head (fp8
        DoubleRow: both ftT and wv are fp8, contraction 256/pass)."""
        for i, (c0, c1) in enumerate(src_sel):
            rows = c1 - c0
            vt = vtiles[i]
            vt3 = vt.rearrange("r (h x) -> r h x", x=65)
            for half in range(2):
                pv = ps_a.tile([P, 512], F32, name="psa")[:rows, :]
                for c in range(4):
                    nc.tensor.matmul(
                        pv, src3d[:, 2 * c:2 * c + 2, c0:c1],
                        w3[:, 2 * c:2 * c + 2,
                           half * 512:(half + 1) * 512],
                        start=(c == 0), stop=(c == 3), perf_mode=DROW)
                src3 = pv.rearrange("r (h d) -> r h d", d=64)
                nc.scalar.activation(
                    out=vt3[:rows, half * 8:(half + 1) * 8, 0:64], in_=src3,
                    func=AF.Copy)
            nc.vector.memset(vt3[:rows, :, 64:65], 1.0)

    # ================= layers =================
    for l in range(nlayers):
        layernorm(yT, xn8, f"ln1_{l}")
        dma(chunked(t["ag_in"][l]), xn83[:])
        nc.gpsimd.collective_compute(
            "AllGather", ALU.bypass, replica_groups=t["RG"],
            ins=[t["ag_in"][l][:]], outs=[t["ag_out"][l][:]],
        )
        dma(xh03[:, :, :TO], chunked(t["ag_out"][l][0]))
        dma(xh13[:, :, :TO], chunked(t["ag_out"][l][1]))

        # ---- cross attention (overlaps the AllGather) ----
        wq = load_w(t["wqm"], l, F8E5)[8]
        for m in range(8):
            pt = ps_a.tile([P, 512], F32, name="psa")[:, :TO]
            for c in range(4):
                nc.tensor.matmul(pt, wq[:, 2 * c:2 * c + 2, m * P:(m + 1) * P],
                                 xn83[:, 2 * c:2 * c + 2, :],
                                 start=(c == 0), stop=(c == 3), perf_mode=DROW)
            nc.vector.tensor_copy(out=qm[m][:], in_=pt)
        wv = load_w(t["wvm"], l, F8E5)[8]
        vm = [kvp.tile([P, 16 * 65], BF16, name=f"vm{i}") for i in range(kp)]
        make_v(wv, ftT3, [(i * P, (i + 1) * P) for i in range(kp)], vm)
        wkm_t = load_w(t["wkm"], l, F8E5)[8]
        km = [kvp.tile([P, S], BF16, name=f"km{i}") for i in range(8)]
        kcols = [(0, min(512, kp * P))]
        if kp * P > 512:
            kcols.append((512, kp * P))
        for (s0, s1) in kcols:
            for m in range(8):
                pt = ps_a.tile([P, 512], F32, name="psa")[:, :s1 - s0]
                for c in range(4):
                    nc.tensor.matmul(
                        pt, wkm_t[:, 2 * c:2 * c + 2, m * P:(m + 1) * P],
                        ftT3[:, 2 * c:2 * c + 2, s0:s1],
                        start=(c == 0), stop=(c == 3), perf_mode=DROW)
                nc.scalar.activation(out=km[m][:, s0:s1], in_=pt,
                                     func=AF.Copy)
        # ---- self-attn projections, emitted as filler units inside the
        # (ACT-bound) cross-attention head loop so the PE queue never drains.
        rpend = []

        def flush_rope(limit=2):
            while len(rpend) > limit:
                st, c0, c1, stab, dst = rpend.pop(0)
                rope2(st, c0, c1, stab, dst)

        wcur = {}
        kt = [kvp.tile([P, TK], BF16, name=f"kt{i}") for i in range(8)]
        vt_tiles = [kvp.tile([P, 16 * 65], BF16, name=f"vt{i}") for i in range(4)]
        vt_tiles.append(kvp.tile([2, 16 * 65], BF16, name="vt4"))
        units = []

        def u_load(key, dram):
            def f():
                wcur[key] = load_w(dram, l, F8E5)[8]
            return f

        def u_qt(m):
            def f():
                w3 = wcur["qt"]
                pt = ps_a.tile([P, 512], F32, name="psa")[:, :TO]
                for c in range(4):
                    nc.tensor.matmul(
                        pt, w3[:, 2 * c:2 * c + 2, m * P:(m + 1) * P],
                        xn83[:, 2 * c:2 * c + 2, :],
                        start=(c == 0), stop=(c == 3), perf_mode=DROW)
                rpend.append((rope1(pt, 0, TO, cq), 0, TO, sq, qt[m][:]))
                flush_rope()
            return f

        def u_kt(m, src3d, c0, cw):
            def f():
                w3 = wcur["kt"]
                pt = ps_a.tile([P, 512], F32, name="psa")[:, :cw]
                for c in range(4):
                    nc.tensor.matmul(
                        pt, w3[:, 2 * c:2 * c + 2, m * P:(m + 1) * P],
                        src3d[:, 2 * c:2 * c + 2, :cw],
                        start=(c == 0), stop=(c == 3), perf_mode=DROW)
                rpend.append((rope1(pt, c0, c0 + cw, ck), c0, c0 + cw, sk,
                              kt[m][:, c0:c0 + cw]))
                flush_rope()
            return f

        def u_vt(i, src3d, c0, c1, half):
            def f():
                rows = c1 - c0
                w3 = wcur["vt"]
                vt3 = vt_tiles[i].rearrange("r (h x) -> r h x", x=65)
                pv = ps_a.tile([P, 512], F32, name="psa")[:rows, :]
                # xh stationary: TOP-padded stride makes the pair-step
                # 16B-aligned, so DoubleRow (256-deep fp8) applies
                for c in range(4):
                    nc.tensor.matmul(
                        pv, src3d[:, 2 * c:2 * c + 2, c0:c1],
                        w3[:, 2 * c:2 * c + 2, half * 512:(half + 1) * 512],
                        start=(c == 0), stop=(c == 3), perf_mode=DROW)
                src3 = pv.rearrange("r (h d) -> r h d", d=64)
                nc.vector.tensor_copy(
                    out=vt3[:rows, half * 8:(half + 1) * 8, 0:64], in_=src3)
                if half == 1:
                    nc.vector.memset(vt3[:rows, :, 64:65], 1.0)
            return f

        units.append(u_load("qt", t["wqt"]))
        for m in range(8):
            units.append(u_qt(m))
        units.append(u_load("kt", t["wkt"]))
        for m in range(8):
            for (src3d, c0, cw) in [(xh03, 0, 256), (xh13, 256, TO)]:
                units.append(u_kt(m, src3d, c0, cw))
        units.append(u_load("vt", t["wvt"]))
        for i, (src3d, c0, c1) in enumerate(
                [(xh03, 0, 128), (xh03, 128, 256), (xh13, 0, 128),
                 (xh13, 128, 256), (xh13, 256, TO)]):
            for half in range(2):
                units.append(u_vt(i, src3d, c0, c1, half))
        units.append(lambda: flush_rope(0))

        def cross_filler(n=2):
            for _ in range(n):
                if units:
                    units.pop(0)()

        attention(qm, km, vm,
                  [(i, i * P, (i + 1) * P, 0, TO) for i in range(kp)],
                  osb, osb8, "mem", f"x{l}", filler=cross_filler,
                  clean=kclean)
        while units:
            units.pop(0)()

        # causal-trimmed pieces: key slots [r0A|r0B|r1A|r1B|tail] vs own
        # query cols; slot 0 must span the full query range (start=True).
        # mask multiplies cover only the (mc0, mc1) sub-ranges with zeros.
        attention(qt, kt, vt_tiles,
                  [(0, 0, 128, 0, TO, 0, 128), (1, 128, 256, 128, TO, 128, TO),
                   (2, 256, 384, 0, TO, 0, 128), (3, 384, 512, 128, TO, 128, TO),
                   (4, 512, 514, 256, TO, 256, TO)],
                  osb2, osb28, "self", f"s{l}",
                  groups=[(0, 2), (1, 3), (4,)])

        # ---- combined output projections (DVE: single add into y) ----
        wo1 = load_w(t["wom"], l, F8E5)[8]
        wo2 = load_w(t["wot"], l, F8E5)[8]
        for m in range(8):
            pt = ps_a.tile([P, 512], F32, name="psa")[:, :TO]
            for c in range(4):
                nc.tensor.matmul(
                    pt, wo1[:, 2 * c:2 * c + 2, m * P:(m + 1) * P],
                    osb83[:, 2 * c:2 * c + 2, :],
                    start=(c == 0), stop=False, perf_mode=DROW)
            for c in range(4):
                nc.tensor.matmul(
                    pt, wo2[:, 2 * c:2 * c + 2, m * P:(m + 1) * P],
                    osb283[:, 2 * c:2 * c + 2, :],
                    start=False, stop=(c == 3), perf_mode=DROW)
            nc.vector.tensor_tensor(out=yT[m][:], in0=pt, in1=yT[m][:],
                                    op=ALU.add)

        # ---- FFN (fp8 DoubleRow: h = gelu(xn8 @ W1sub), y += W2sub^T h) ----
        layernorm(yT, xn8, f"ln2_{l}")
        for sub in range(4):
            w1b = w1p.tile([P, 8, 1024], F8E5, name="w1p")
            wdma(w1b[:], chunked(t["w1"][l, :, sub * 1024:(sub + 1) * 1024]))
            hsub = []
            for mc in range(4):
                hp = hbf_p.tile([P, 2, TO], F8E4, name="hbf")
                sp2 = ps_p.tile([P, 2, 512], F32, name="psp")
                for j in range(2):
                    m = 2 * mc + j
                    for c in range(4):
                        nc.tensor.matmul(
                            sp2[:, j, :TO],
                            w1b[:, 2 * c:2 * c + 2, m * P:(m + 1) * P],
                            xn83[:, 2 * c:2 * c + 2, :],
                            start=(c == 0), stop=(c == 3), perf_mode=DROW)
                nc.scalar.activation(out=hp[:], in_=sp2[:, :, :TO],
                                     func=AF.Gelu)
                hsub.append(hp)
            w2b = w1p.tile([P, 8, 1024], F8E5, name="w1p")
            wdma(w2b[:], chunked(t["w2"][l, sub * 1024:(sub + 1) * 1024, :]))
            for m in range(8):
                pt = ps_a.tile([P, 512], F32, name="psa")[:, :TO]
                for c in range(4):
                    nc.tensor.matmul(
                        pt, w2b[:, 2 * c:2 * c + 2, m * P:(m + 1) * P],
                        hsub[c][:, :, :],
                        start=(c == 0), stop=(c == 3), perf_mode=DROW)
                nc.vector.tensor_tensor(out=yT[m][:], in0=pt, in1=yT[m][:],
                                        op=ALU.add)

    # ================= head =================
    # logits computed [128-vocab-tile, TO] (vocab on partitions): wout is
    # the DoubleRow stationary, xn8 the moving operand; exp'd tiles are
    # reduced over vocab partitions by an accumulating ones-matmul.
    layernorm(yT, xn, "lnf")
    for k in range(8):
        nc.scalar.activation(out=xn8[k][:], in_=xn[k][:], func=AF.Copy)
    sep = ps_o.tile([1, TO], F32, name="psb_o")
    for nvp in range(8):
        wb = w1p.tile([P, 8, 1024], F8E5, name="w1p")
        wdma(wb[:], chunked(t["wout"][:, nvp * 1024:(nvp + 1) * 1024]))
        for vp in range(4):
            sp2 = ps_p.tile([P, 2, 512], F32, name="psp")
            esc2 = pbf.tile([P, 2, TO], BF16, name="pbf2", bufs=4)
            for j in range(2):
                vsub = vp * 2 + j
                for c in range(4):
                    nc.tensor.matmul(
                        sp2[:, j, :TO],
                        wb[:, 2 * c:2 * c + 2, vsub * P:(vsub + 1) * P],
                        xn83[:, 2 * c:2 * c + 2, :],
                        start=(c == 0), stop=(c == 3), perf_mode=DROW)
            nc.scalar.activation(out=esc2[:], in_=sp2[:, :, :TO], func=AF.Exp)
            for j in range(2):
                vi = nvp * 8 + vp * 2 + j
                nc.tensor.matmul(sep[:], ones_k[:], esc2[:, j, :],
                                 start=(vi == 0), stop=(vi == 63))
    ses = vrow.tile([1, TO], F32, name="vrow")
    nc.scalar.activation(out=ses[:], in_=sep[:], func=AF.Copy)
    dma(t["out_se"][:, :], ses[:])

    # target logit: tl = sum_c wtgt * yf (bf16-rounded, matching logits path)
    tlacc = scr.tile([P, TO], F32, name="tlacc", bufs=1)
    for k in range(8):
        xf = scr.tile([P, TO], F32, name="scr")
        nc.vector.tensor_tensor(out=xf[:], in0=xn[k][:], in1=wtg[k][:],
                                op=ALU.mult)
        if k == 0:
            nc.vector.tensor_copy(out=tlacc[:], in_=xf[:])
        else:
            nc.vector.tensor_add(out=tlacc[:], in0=tlacc[:], in1=xf[:])
    tlp = ps_o.tile([1, TO], F32, name="psb_o")
    nc.tensor.matmul(tlp[:], ones_kf[:], tlacc[:], start=True, stop=True)
    tls = vrow.tile([1, TO], F32, name="vrow")
    nc.scalar.activation(out=tls[:], in_=tlp[:], func=AF.Copy)
    dma(t["out_tl"][:, :], tls[:])


# ======================= host side =======================

def _host_inputs(features, targets, input_lengths, target_lengths, wte, ln1_w,
                 Wq_m, Wk_m, Wv_m, Wo_m, Wq_t, Wk_t, Wv_t, Wo_t, ln2_w, W1,
                 W2, lnf_w, Wout, nlayers):
    bf = ml_dtypes.bfloat16
    f32 = np.float32
    features = np.asarray(features, f32)
    targets = np.asarray(targets).astype(np.int64)
    input_lengths = np.asarray(input_lengths).astype(np.int64)
    target_lengths = np.asarray(target_lengths).astype(np.int64)
    ln1_w = np.asarray(ln1_w, f32)
    ln2_w = np.asarray(ln2_w, f32)
    lnf_w = np.asarray(lnf_w, f32)

    n = targets.shape[0]
    prompt = np.concatenate(
        [np.full((n, 1), STX, np.int64), targets], axis=1)  # [N, T]
    tgt = np.concatenate([targets, np.zeros((n, 1), np.int64)], axis=1)
    tgt[np.arange(n), target_lengths] = ETX

    perm = _rope_perm()
    pos_k = _key_global_idx()
    f8 = 1.0 / np.sqrt(np.float32(D))

    def cast(x):
        return np.ascontiguousarray(np.asarray(x, f32)).astype(bf)

    e4 = ml_dtypes.float8_e4m3
    e5 = ml_dtypes.float8_e5m2

    def cast8(x):
        return np.ascontiguousarray(np.asarray(x, f32)).astype(e5)

    sharedw = {
        "wqm": cast8(np.asarray(Wq_m, f32)[:nlayers] * ln1_w[:nlayers, :, None] * f8),
        "wkm": cast8(np.asarray(Wk_m, f32)[:nlayers]),
        "wvm": cast8(np.asarray(Wv_m, f32)[:nlayers]),
        "wom": cast8(np.asarray(Wo_m, f32)[:nlayers]),
        "wqt": cast8((np.asarray(Wq_t, f32)[:nlayers] * ln1_w[:nlayers, :, None]
                      * f8)[:, :, perm]),
        "wkt": cast8((np.asarray(Wk_t, f32)[:nlayers]
                      * ln1_w[:nlayers, :, None])[:, :, perm]),
        "wvt": cast8(np.asarray(Wv_t, f32)[:nlayers] * ln1_w[:nlayers, :, None]),
        "wot": cast8(np.asarray(Wo_t, f32)[:nlayers]),
        "w1": cast8(np.asarray(W1, f32)[:nlayers] * ln2_w[:nlayers, :, None]),
        "w2": cast8(np.asarray(W2, f32)[:nlayers]),
        "ptsw": _pswap().astype(bf),
    }
    wout_f = np.asarray(Wout, f32) * lnf_w[:, None]
    wout_bf = (np.asarray(Wout, f32) * lnf_w[:, None]).astype(e5)
    sharedw["wout"] = wout_bf

    y0_all = np.asarray(wte, f32)[prompt]  # [N, T, C]

    in_maps, meta = [], []
    for core in range(8):
        nb, r = core // 2, core % 2
        own = _own_global_idx(r)
        ownpos = np.maximum(own, 0)
        y0T = np.where(own[None, :] >= 0, y0_all[nb][ownpos].T, 0.0).astype(bf)
        cosq_, sinq_ = _rope_tables(own)
        cosk_, sink_ = _rope_tables(pos_k)
        mbig, mtail = _self_masks(r)
        memmask = np.where(np.arange(S) < input_lengths[nb], 0.0,
                           NEG).astype(f32)[:, None]
        mm01 = (np.arange(S) < input_lengths[nb]).astype(f32)[:, None]
        padmask = (own >= 0).astype(f32)[None, :]
        wtgt = np.where(own[None, :] >= 0,
                        wout_bf.astype(f32)[:, tgt[nb][ownpos]], 0.0).astype(bf)
        im = {
            "y0": y0T,
            "featT": np.ascontiguousarray(features[nb].T).astype(e4),
            "memmask": memmask,
            "mm01": mm01,
            "mbig": mbig.astype(bf), "mtail": mtail.astype(bf),
            "padmask": padmask,
            "cosq": cosq_.astype(bf), "sinq": sinq_.astype(bf),
            "cosk": cosk_.astype(bf), "sink": sink_.astype(bf),
            "wtgt": wtgt,
        }
        im.update(sharedw)
        in_maps.append(im)
        valid = np.where(own >= 0, (tgt[nb][ownpos] != 0), False)
        meta.append((nb, own, valid))
    return in_maps, meta


def kernel(features, targets, input_lengths, target_lengths, wte, ln1_w,
           Wq_m, Wk_m, Wv_m, Wo_m, Wq_t, Wk_t, Wv_t, Wo_t, ln2_w, W1, W2,
           lnf_w, Wout):
    nlayers = L
    # specialize the program to the live cross-attn key range: keys beyond
    # max(input_lengths) rounded up to 128 are masked for every sample
    kp = int(min(8, max(1, -(-int(np.max(input_lengths)) // 128))))
    kclean = int(min(kp, max(0, int(np.min(input_lengths)) // 128)))
    key = (nlayers, kp, kclean)
    if key not in _prog_cache:
        _prog_cache[key] = _build_program(nlayers, kp, kclean)
    nc = _prog_cache[key]

    in_maps, meta = _host_inputs(
        features, targets, input_lengths, target_lengths, wte, ln1_w,
        Wq_m, Wk_m, Wv_m, Wo_m, Wq_t, Wk_t, Wv_t, Wo_t, ln2_w, W1, W2,
        lnf_w, Wout, nlayers)

    res = run_bass_kernel_spmd(nc, in_maps, core_ids=list(range(8)))
    globals()["LAST_RESULTS"] = res

    num, den = 0.0, 0.0
    for core in range(8):
        r = res.results[core]
        _, own, valid = meta[core]
        sumexp = r["out_se"][0]
        tl = r["out_tl"][0]
        nll = np.log(np.maximum(sumexp, 1e-300)) - tl
        num += float(np.sum(nll[valid]))
        den += float(np.sum(valid))
    return np.float32(num / max(den, 1.0))



# revision 37
# speedup vs baseline: 1.2371x; 1.0565x over previous
"""Trainium2 Bass kernel for nn_CTCAttentionDecoder.

12-layer transformer decoder (cross-attn over encoder memory + causal
self-attn with rotary embeddings + FFN) -> LM head -> masked NLL loss.

Parallelization: 8 NeuronCores = 4 pairs (one batch sample each); within a
pair, decoder tokens are split between the two cores (interleaved 128-token
chunks to balance causal attention work). K/V are computed redundantly on
both cores so the only per-layer communication is a single pair-AllGather of
the layer-normed activations (fp8). The LM head runs with full vocab on
each core for its own tokens; the host combines 8 per-token partial results
into the scalar loss.

Precision: residual stream and softmax/layernorm statistics in fp32;
attention scores/AV in bf16; all weight projections (QKVO, FFN, K/V-mem,
LM head) run in fp8 (e5m2 weights x e4m3 activations) with fp32 PSUM
accumulation, using DoubleRow 256-deep contraction where the stationary
operand's pair-step is 16B-aligned. Final rel err vs the fp32 reference is
~1e-3, against a 2e-2 tolerance.

Scheduling: engine queues execute in emission order, so the build software-
pipelines everything: self-attn projections are emitted as filler units
inside the ACT-bound cross-attention head loop, rope's swap-matmul and the
softmax normalization run 1-2 chunks/heads behind their producers, and
causally-dead score/exp/mask work is skipped via per-piece query ranges.
"""

import os

import numpy as np
import ml_dtypes

import concourse.bacc as bacc
import concourse.mybir as mybir
import concourse.tile as tile
from concourse.bass_utils import run_bass_kernel_spmd

F32 = mybir.dt.float32
BF16 = mybir.dt.bfloat16
F8E4 = mybir.dt.float8e4
F8E5 = mybir.dt.float8e5
DROW = mybir.MatmulPerfMode.DoubleRow
AF = mybir.ActivationFunctionType
ALU = mybir.AluOpType

N, S, T0, C, H, D, NLAYERS, V, FF = 4, 1024, 512, 1024, 16, 64, 12, 8192, 4096
T = T0 + 1  # 513
TO = 258  # own tokens per core (incl. pad columns)
TOP = 272  # padded xh stride (16B-aligned pair-step for DoubleRow)
TK = 514  # key slots: [c0 | c3 | c1 | c2 | t512 | pad]
STX, ETX = 3, 4
NEG = -1e30

L = int(os.environ.get("K_LAYERS", str(NLAYERS)))

_prog_cache = {}


def _own_global_idx(r):
    """Global token index per own column; -1 for pad columns."""
    if r == 0:
        return np.concatenate([np.arange(0, 128), np.arange(384, 512), [-1, -1]])
    return np.concatenate([np.arange(128, 256), np.arange(256, 384), [512, -1]])


def _key_global_idx():
    """Global token index per key slot; -1 for the pad slot."""
    return np.concatenate(
        [np.arange(0, 128), np.arange(384, 512), np.arange(128, 256),
         np.arange(256, 384), [512, -1]]
    )


def _rope_tables(pos, rows=128):
    """cos/sin tables [rows, len(pos)]; row i uses theta_(i%32)."""
    th = (10000.0 ** (-2.0 * np.arange(32) / D))  # [32]
    ang = th[:, None] * np.maximum(pos, 0)[None, :].astype(np.float64)
    cos = np.cos(ang).astype(np.float32)
    sin = np.sin(ang).astype(np.float32)
    reps = rows // 32
    return np.tile(cos, (reps, 1)), np.tile(sin, (reps, 1))


def _self_masks(r):
    """Causal masks, multiplicative {0, 1}: [4, 128, TO] big chunks + [2, TO]
    tail. Applied to exp(scores) (p = exp(s) * m), so no NEG bias needed."""
    own = _own_global_idx(r)  # [TO]
    key = _key_global_idx()  # [TK]
    big = np.zeros((4, 128, TO), np.float32)
    for kc in range(4):
        kg = key[kc * 128:(kc + 1) * 128]
        big[kc] = (kg[:, None] <= own[None, :]).astype(np.float32)
    tail = (key[512:514, None] <= own[None, :]).astype(np.float32)
    tail[key[512:514] < 0, :] = 0.0  # pad key slot: never attended
    # Pad query columns: allow everything so rowsum > 0 (their output is
    # garbage-but-finite and never read; a fully-masked row gives 0/0 NaN
    # that would pollute real tokens through later layers).
    pad_q = own < 0
    big[:, :, pad_q] = 1.0
    tail[:, pad_q] = 1.0
    return big, tail


def _rope_perm():
    """Column permutation de-interleaving rotary pairs within each head."""
    p = np.arange(C).reshape(H, D)
    newd = np.concatenate([np.arange(0, D, 2), np.arange(1, D, 2)])
    return p[:, newd].reshape(-1)


def _pswap():
    """PT [128,128] with qswap = PT.T @ q: out[r]=-q[r+32], out[r+32]=q[r]."""
    PT = np.zeros((128, 128), np.float32)
    for b in range(0, 128, 64):
        for i in range(32):
            PT[b + 32 + i, b + i] = -1.0
            PT[b + i, b + 32 + i] = 1.0
    return PT


def _build_program(nlayers, kp=8, kclean=0):
    """kp: number of live 128-key pieces for cross-attention (ceil(max
    input_length / 128)); keys >= kp*128 are masked for every sample so
    their K/V/scores are skipped entirely. kclean: pieces < kclean are
    fully inside every sample's input_length, so their exp needs no mask
    bias (enables paired two-bank exps)."""
    nc = bacc.Bacc("TRN2", num_devices=8)

    def din(name, shape, dtype=BF16):
        return nc.dram_tensor(name, shape, dtype, kind="ExternalInput")

    t = {}
    t["y0"] = din("y0", [C, TO], BF16)
    t["featT"] = din("featT", [C, S], F8E4)
    t["memmask"] = din("memmask", [S, 1], F32)
    t["mm01"] = din("mm01", [S, 1], F32)
    t["mbig"] = din("mbig", [4, 128, TO], BF16)
    t["mtail"] = din("mtail", [2, TO], BF16)
    t["cosq"] = din("cosq", [128, TO], BF16)
    t["sinq"] = din("sinq", [128, TO], BF16)
    t["cosk"] = din("cosk", [128, TK], BF16)
    t["sink"] = din("sink", [128, TK], BF16)
    t["wtgt"] = din("wtgt", [C, TO])
    t["ptsw"] = din("ptsw", [128, 128])
    for nm in ["wqm", "wom", "wqt", "wkt", "wvt", "wot", "wkm", "wvm"]:
        t[nm] = din(nm, [nlayers, C, C], F8E5)
    t["w1"] = din("w1", [nlayers, C, FF], F8E5)
    t["w2"] = din("w2", [nlayers, FF, C], F8E5)
    t["wout"] = din("wout", [C, V], F8E5)

    t["out_se"] = nc.dram_tensor("out_se", [1, TO], F32, kind="ExternalOutput")
    t["out_tl"] = nc.dram_tensor("out_tl", [1, TO], F32, kind="ExternalOutput")

    t["ag_in"] = [nc.dram_tensor(f"agi{l}", [C, TO], F8E4, kind="Internal")
                  for l in range(nlayers)]
    t["ag_out"] = [nc.dram_tensor(f"ago{l}", [2, C, TO], F8E4, kind="Internal")
                   for l in range(nlayers)]
    t["RG"] = [[0, 1], [2, 3], [4, 5], [6, 7]]

    with tile.TileContext(nc) as tc:
        import contextlib
        with contextlib.ExitStack() as ctx:
            with nc.allow_low_precision(
                    reason="bf16 softmax denominators / LN stats are within "
                           "the 2e-2 output tolerance"):
                _build_body(nc, tc, nlayers, t, ctx, kp, kclean)
    nc.finalize()
    return nc


def _build_body(nc, tc, nlayers, t, ctx, kp, kclean):
    P = 128
    ec = ctx.enter_context
    persist = ec(tc.tile_pool(name="persist", bufs=1))
    wk = ec(tc.tile_pool(name="wk", bufs=2))     # [128,8,1024] bf16 weight mats
    w1p = ec(tc.tile_pool(name="w1p", bufs=8))   # [128,8,1024] bf16 ffn/wout
    pbf = ec(tc.tile_pool(name="pbf", bufs=4))   # [128,TO] bf16 exp'd scores
    scr = ec(tc.tile_pool(name="scr", bufs=3))   # [128,TO] fp32 scratch
    scrw = ec(tc.tile_pool(name="scrw", bufs=3))  # [128,512] scratch
    vrow = ec(tc.tile_pool(name="vrow", bufs=4))  # [1,TO] fp32 rows
    vrb = ec(tc.tile_pool(name="vrb", bufs=6))   # [1,TO] bf16 rows
    kvp = ec(tc.tile_pool(name="kvp", bufs=1))   # per-layer kv tiles
    hbf_p = ec(tc.tile_pool(name="hbf_p", bufs=8))  # ffn hidden tiles

    # 8 PSUM banks: ps_p = two 2-bank "pair" tiles (scores/gelu/head exp in
    # [128,2,512] so one ACT instruction covers two pieces), ps_a = two
    # 1-bank projection tiles, ps_o = two 1-bank accumulators (oaug / LN
    # sums; at most 2 live by construction).
    ps_p = ec(tc.tile_pool(name="ps_p", bufs=2, space="PSUM"))
    ps_a = ec(tc.tile_pool(name="ps_a", bufs=2, space="PSUM"))
    ps_o = ec(tc.tile_pool(name="ps_o", bufs=2, space="PSUM"))

    def pt3(nm, n, w, dtype):
        big = persist.tile([P, n, w], dtype, name=nm)
        return big, [big[:, i, :] for i in range(n)]

    yT3, yT = pt3("yT", 8, TO, BF16)
    xn3, xn = pt3("xn", 8, TO, BF16)
    xn83, xn8 = pt3("xn8", 8, TO, F8E4)
    # xh tiles padded to TOP-stride so DoubleRow can use them as the
    # stationary operand (pair-step must be a multiple of 16 bytes)
    xh03 = persist.tile([P, 8, TOP], F8E4, name="xh0")
    xh13 = persist.tile([P, 8, TOP], F8E4, name="xh1")
    ftT3, ftT = pt3("ftT", 8, S, F8E4)
    qm3, qm = pt3("qm", 8, TO, BF16)
    qt3, qt = pt3("qt", 8, TO, BF16)
    osb3, osb = pt3("osb", 8, TO, BF16)   # cross attn o
    osb83, osb8 = pt3("osb8", 8, TO, F8E4)
    osb283, osb28 = pt3("osb28", 8, TO, F8E4)
    osb23, osb2 = pt3("osb2", 8, TO, BF16)  # self attn o
    mmask3, mmask = pt3("mmask", 8, 1, F32)
    mm013, mm01t = pt3("mm01", 8, 1, F32)
    mbig3, mbig_t = pt3("mbigt", 4, TO, BF16)
    mtail_t = persist.tile([2, TO], BF16, name="mtailt")
    cq = persist.tile([P, TO], BF16, name="cq")
    sq = persist.tile([P, TO], BF16, name="sq")
    ck = persist.tile([P, TK], BF16, name="ck")
    sk = persist.tile([P, TK], BF16, name="sk")
    wtg3, wtg = pt3("wtg", 8, TO, BF16)
    ptw = persist.tile([P, P], BF16, name="ptw")
    ones_k = persist.tile([P, 1], BF16, name="ones_k")
    ones_kf = persist.tile([P, 1], F32, name="ones_kf")
    ones_r64 = persist.tile([1, 64], BF16, name="ones_r64")
    ones_r128 = persist.tile([1, P], BF16, name="ones_r128")
    epsr = persist.tile([1, 1], F32, name="epsr")

    dma = nc.sync.dma_start
    wdma = nc.gpsimd.dma_start

    def chunked(dr, p=P):
        return dr.rearrange("(k p) x -> p k x", p=p)

    dma(yT3[:], chunked(t["y0"]))
    dma(ftT3[:], chunked(t["featT"]))
    dma(mmask3[:], chunked(t["memmask"]))
    dma(mm013[:], chunked(t["mm01"]))
    dma(wtg3[:], chunked(t["wtgt"]))
    dma(mbig3[:], t["mbig"].rearrange("c p t -> p c t"))
    dma(mtail_t[:], t["mtail"][:, :])
    dma(cq[:], t["cosq"][:, :])
    dma(sq[:], t["sinq"][:, :])
    dma(ck[:], t["cosk"][:, :])
    dma(sk[:], t["sink"][:, :])
    dma(ptw[:], t["ptsw"][:, :])
    nc.vector.memset(ones_k[:], 1.0)
    nc.vector.memset(ones_kf[:], 1.0)
    nc.vector.memset(ones_r64[:], 1.0)
    nc.vector.memset(ones_r128[:], 1.0)
    nc.vector.memset(epsr[:], 1e-5)

    def layernorm(src_tiles, out_tiles, tag):
        """out (bf16) = (src - mu)/sqrt(var+eps); src tiles are bf16.

        Stat matmuls read the bf16 residual directly; rsqrt is
        exp(-0.5*ln(var+eps)) so only the exp/ln ACT table set is needed.
        """
        sum1 = ps_o.tile([1, TO], F32, name="psb_o")
        sum2 = ps_o.tile([1, TO], F32, name="psb_o")
        # emit the squares first so the PE sum chain streams without waiting
        # per-k on ACT (engine queues execute in emission order)
        ysqs = []
        for k in range(8):
            ysq = pbf.tile([P, TO], BF16, name="lnb", bufs=12)
            nc.scalar.square(out=ysq[:], in_=src_tiles[k][:])
            ysqs.append(ysq)
        for k in range(8):
            nc.tensor.matmul(sum1[:], ones_k[:], src_tiles[k][:],
                             start=(k == 0), stop=(k == 7))
            nc.tensor.matmul(sum2[:], ones_k[:], ysqs[k][:],
                             start=(k == 0), stop=(k == 7))
        mub_r = vrb.tile([1, TO], BF16, name="vrb")
        nc.scalar.activation(out=mub_r[:], in_=sum1[:], func=AF.Copy,
                             scale=1.0 / C)
        mub = ps_a.tile([P, 512], F32, name="psa")[:, :TO]
        nc.tensor.matmul(mub, ones_r128[:], mub_r[:], start=True, stop=True)
        mu = vrow.tile([1, TO], F32, name="vrow")
        nc.scalar.mul(out=mu[:], in_=sum1[:], mul=1.0 / C)
        musq = vrow.tile([1, TO], F32, name="vrow")
        nc.vector.tensor_mul(out=musq[:], in0=mu[:], in1=mu[:])
        var = vrow.tile([1, TO], F32, name="vrow")
        nc.scalar.activation(out=var[:], in_=sum2[:], func=AF.Copy, scale=1.0 / C)
        nc.vector.tensor_sub(out=var[:], in0=var[:], in1=musq[:])
        lnv = vrow.tile([1, TO], F32, name="vrow")
        nc.scalar.activation(out=lnv[:], in_=var[:], func=AF.Ln, bias=epsr[:])
        rinv_b = vrb.tile([1, TO], BF16, name="vrb")
        nc.scalar.activation(out=rinv_b[:], in_=lnv[:], func=AF.Exp, scale=-0.5)
        rsb = ps_a.tile([P, 512], F32, name="psa")[:, :TO]
        nc.tensor.matmul(rsb, ones_r128[:], rinv_b[:], start=True, stop=True)
        mub_s = scrw.tile([P, TO], BF16, name="lnbc", bufs=2)
        nc.vector.tensor_copy(out=mub_s[:], in_=mub)
        rsb_s = scrw.tile([P, TO], BF16, name="lnbc", bufs=2)
        nc.vector.tensor_copy(out=rsb_s[:], in_=rsb)
        for k in range(8):
            d = pbf.tile([P, TO], BF16, name="lnd", bufs=2)
            nc.vector.tensor_sub(out=d[:], in0=src_tiles[k][:], in1=mub_s[:])
            nc.vector.tensor_tensor(out=out_tiles[k][:], in0=d[:], in1=rsb_s[:],
                                    op=ALU.mult)

    def load_w(dram, l, dt=BF16):
        # fp8 weight blocks ride in the w1p pool (same 16KB slot budget)
        wt = (wk.tile([P, 8, C], BF16, name="wk") if dt == BF16
              else w1p.tile([P, 8, C], dt, name="w1p"))
        wdma(wt[:], chunked(dram[l]))
        sl = [wt[:, k, :] for k in range(8)]
        sl.append(wt)
        return sl

    def proj_T(wtiles, rhs_tiles, consume):
        for m in range(8):
            pt = ps_a.tile([P, 512], F32, name="psa")[:, :TO]
            for k in range(8):
                nc.tensor.matmul(pt, wtiles[k][:, m * P:(m + 1) * P],
                                 rhs_tiles[k][:, :TO],
                                 start=(k == 0), stop=(k == 7))
            consume(pt, m)

    def rope1(pt, c0, c1, ctab):
        """Stage 1: pb = bf16(pt) on ACT; a = pb*cos on Pool (both engines
        have slack; DVE is the critical engine)."""
        w = c1 - c0
        pb = scrw.tile([P, TO], BF16, name="ropep")[:, :w]
        nc.vector.tensor_copy(out=pb, in_=pt)
        a = scrw.tile([P, TO], BF16, name="ropea")[:, :w]
        nc.vector.tensor_tensor(out=a, in0=pb, in1=ctab[:, c0:c1], op=ALU.mult)
        return a, pb

    def rope2(st, c0, c1, stab, out_bf):
        """Stage 2 (emit one chunk later): out = a + (PT.T@pb)*sin."""
        a, pb = st
        w = c1 - c0
        swp = ps_a.tile([P, 512], F32, name="psa")[:, :w]
        nc.tensor.matmul(swp, ptw[:], pb, start=True, stop=True)
        b = scrw.tile([P, TO], BF16, name="ropeb", bufs=2)[:, :w]
        nc.vector.tensor_tensor(out=b, in0=swp, in1=stab[:, c0:c1], op=ALU.mult)
        nc.vector.tensor_tensor(out=out_bf, in0=a, in1=b, op=ALU.add)

    def attention(q_tiles, k_tiles, v_tiles, pieces, o_tiles, o8_tiles,
                  mask_mode, tag, filler=None, clean=0, groups=None):
        """pieces: (kci, kr0, kr1, qc0, qc1) — key-slot rows x query cols.

        Piece 0 must span the full query range (its start=True write fills
        oaug's has_written bits for the whole accumulation region). Self
        masks are multiplicative {0,1} bf16 applied to exp(scores).

        Scores land in 2-bank ps_p pair tiles; a same-width group whose
        pieces need no exp-bias (mem pieces < `clean`, or any self pieces)
        is exp'd by ONE ACT instruction over a [128,2,w] AP — the per-
        instruction ACT bubble is ~40% of a 258-wide exp, so pairing is a
        big ACT saving. `groups` orders score/exp work; AV accumulation
        keeps the original piece order.

        `filler()` is invoked once per head to emit independent PE work into
        the queue so the PE never drains while ACT chews on softmax.
        """
        npc = len(pieces)
        if groups is None:
            groups = [tuple(range(i, min(i + 2, npc)))
                      for i in range(0, npc, 2)]

        rcs = {}

        def finalize(h, oaug):
            """Evacuate head h's oaug unnormalized (bf16) + its reciprocal
            softmax denominator; the normalizing multiply runs two heads
            later (norm) so the PE never waits on the DVE reciprocal.
            Copies alternate DVE/ACT to balance the two engines."""
            ht, hb = h // 2, (h % 2) * 64
            nc.scalar.activation(out=o_tiles[ht][hb:hb + 64, :],
                                 in_=oaug[0:64, :], func=AF.Copy)
            rc = vrb.tile([1, TO], BF16, name="vrb")
            nc.vector.reciprocal(out=rc[:], in_=oaug[64:65, :])
            rcs[h] = rc

        def norm(m):
            rb = ps_a.tile([P, 512], F32, name="psa")[:, :TO]
            nc.tensor.matmul(rb[0:64, :], ones_r64[:], rcs.pop(2 * m)[:],
                             start=True, stop=True)
            nc.tensor.matmul(rb[64:128, :], ones_r64[:], rcs.pop(2 * m + 1)[:],
                             start=True, stop=True)
            nc.vector.tensor_tensor(out=o8_tiles[m][:], in0=o_tiles[m][:],
                                    in1=rb, op=ALU.mult)

        pending = []  # (h, oaug) of recent heads, not yet evacuated
        normq = []
        for h in range(H):
            ht, hb = h // 2, (h % 2) * 64
            oaug = ps_o.tile([65, TO], F32, name="psb_o")
            ps = [None] * npc
            for g in groups:
                w0 = pieces[g[0]][4] - pieces[g[0]][3]
                joint = (len(g) == 2
                         and pieces[g[1]][4] - pieces[g[1]][3] == w0)
                sp2 = ps_p.tile([P, 2, 512], F32, name="psp")
                for idx, j in enumerate(g):
                    (kci, kr0, kr1, qc0, qc1) = pieces[j][:5]
                    rows = kr1 - kr0
                    w = qc1 - qc0
                    nc.tensor.matmul(
                        sp2[:rows, idx, :w], k_tiles[ht][hb:hb + 64, kr0:kr1],
                        q_tiles[ht][hb:hb + 64, qc0:qc1],
                        start=True, stop=True, tile_position=(hb, 0),
                    )
                p2 = None
                if joint:
                    p2 = pbf.tile([P, 2, TO], BF16, name="pbf2", bufs=4)
                    nc.scalar.activation(out=p2[:, :, :w0],
                                         in_=sp2[:, :, :w0], func=AF.Exp)
                for idx, j in enumerate(g):
                    (kci, kr0, kr1, qc0, qc1), mrng = pieces[j][:5], pieces[j][5:]
                    rows = kr1 - kr0
                    w = qc1 - qc0
                    if joint:
                        p = p2[:rows, idx, :w]
                        if mask_mode == "mem" and j >= clean:
                            nc.vector.tensor_scalar_mul(
                                out=p, in0=p, scalar1=mm01t[kci][:rows, :])
                    else:
                        p = pbf.tile([P, TO], BF16, name="pbf")[:rows, :w]
                        if mask_mode == "mem":
                            nc.scalar.activation(out=p, in_=sp2[:rows, idx, :w],
                                                 func=AF.Exp,
                                                 bias=mmask[kci][:rows, :])
                        else:
                            nc.scalar.activation(out=p, in_=sp2[:rows, idx, :w],
                                                 func=AF.Exp)
                    if mask_mode != "mem" and mrng:
                        mc0, mc1 = mrng
                        mt = (mbig_t[kci][:rows, mc0:mc1] if kci < 4
                              else mtail_t[0:rows, mc0:mc1])
                        pm = p[:, mc0 - qc0:mc1 - qc0]
                        nc.vector.tensor_tensor(out=pm, in0=pm, in1=mt,
                                                op=ALU.mult)
                    ps[j] = (j, kci, rows, qc0, qc1, p)
            if normq:
                norm(normq.pop(0))
            if len(pending) >= 1:
                finalize(*pending.pop(0))
                if h - 1 >= 1 and (h - 1) % 2 == 1:
                    normq.append((h - 1) // 2)
            for (j, kci, rows, qc0, qc1, p) in ps:
                nc.tensor.matmul(oaug[:, qc0:qc1],
                                 v_tiles[kci][:rows, h * 65:(h + 1) * 65],
                                 p, start=(j == 0), stop=(j == npc - 1))
            pending.append((h, oaug))
            if filler is not None:
                filler()
        for pd in pending:
            finalize(*pd)
        for m in normq:
            norm(m)
        for m in range(8):
            if 2 * m in rcs:
                norm(m)

    def make_v(w3, src3d, src_sel, vtiles):
        """v tiles [rows, 16*65] bf16 with a ones column per head (fp8
        DoubleRow: both ftT and wv are fp8, contraction 256/pass)."""
        for i, (c0, c1) in enumerate(src_sel):
            rows = c1 - c0
            vt = vtiles[i]
            vt3 = vt.rearrange("r (h x) -> r h x", x=65)
            for half in range(2):
                pv = ps_a.tile([P, 512], F32, name="psa")[:rows, :]
                for c in range(4):
                    nc.tensor.matmul(
                        pv, src3d[:, 2 * c:2 * c + 2, c0:c1],
                        w3[:, 2 * c:2 * c + 2,
                           half * 512:(half + 1) * 512],
                        start=(c == 0), stop=(c == 3), perf_mode=DROW)
                src3 = pv.rearrange("r (h d) -> r h d", d=64)
                if half == 0:
                    nc.scalar.activation(
                        out=vt3[:rows, half * 8:(half + 1) * 8, 0:64],
                        in_=src3, func=AF.Copy)
                else:
                    nc.vector.tensor_copy(
                        out=vt3[:rows, half * 8:(half + 1) * 8, 0:64],
                        in_=src3)
            nc.vector.memset(vt3[:rows, :, 64:65], 1.0)

    # ================= layers =================
    for l in range(nlayers):
        layernorm(yT, xn8, f"ln1_{l}")
        dma(chunked(t["ag_in"][l]), xn83[:])
        nc.gpsimd.collective_compute(
            "AllGather", ALU.bypass, replica_groups=t["RG"],
            ins=[t["ag_in"][l][:]], outs=[t["ag_out"][l][:]],
        )
        dma(xh03[:, :, :TO], chunked(t["ag_out"][l][0]))
        dma(xh13[:, :, :TO], chunked(t["ag_out"][l][1]))

        # ---- cross attention (overlaps the AllGather) ----
        wq = load_w(t["wqm"], l, F8E5)[8]
        for m in range(8):
            pt = ps_a.tile([P, 512], F32, name="psa")[:, :TO]
            for c in range(4):
                nc.tensor.matmul(pt, wq[:, 2 * c:2 * c + 2, m * P:(m + 1) * P],
                                 xn83[:, 2 * c:2 * c + 2, :],
                                 start=(c == 0), stop=(c == 3), perf_mode=DROW)
            nc.vector.tensor_copy(out=qm[m][:], in_=pt)
        wv = load_w(t["wvm"], l, F8E5)[8]
        vm = [kvp.tile([P, 16 * 65], BF16, name=f"vm{i}") for i in range(kp)]
        make_v(wv, ftT3, [(i * P, (i + 1) * P) for i in range(kp)], vm)
        wkm_t = load_w(t["wkm"], l, F8E5)[8]
        km = [kvp.tile([P, S], BF16, name=f"km{i}") for i in range(8)]
        kcols = [(0, min(512, kp * P))]
        if kp * P > 512:
            kcols.append((512, kp * P))
        for (s0, s1) in kcols:
            for m in range(8):
                pt = ps_a.tile([P, 512], F32, name="psa")[:, :s1 - s0]
                for c in range(4):
                    nc.tensor.matmul(
                        pt, wkm_t[:, 2 * c:2 * c + 2, m * P:(m + 1) * P],
                        ftT3[:, 2 * c:2 * c + 2, s0:s1],
                        start=(c == 0), stop=(c == 3), perf_mode=DROW)
                if m % 2 == 0:
                    nc.scalar.activation(out=km[m][:, s0:s1], in_=pt,
                                         func=AF.Copy)
                else:
                    nc.vector.tensor_copy(out=km[m][:, s0:s1], in_=pt)
        # ---- self-attn projections, emitted as filler units inside the
        # (ACT-bound) cross-attention head loop so the PE queue never drains.
        rpend = []

        def flush_rope(limit=2):
            while len(rpend) > limit:
                st, c0, c1, stab, dst = rpend.pop(0)
                rope2(st, c0, c1, stab, dst)

        wcur = {}
        kt = [kvp.tile([P, TK], BF16, name=f"kt{i}") for i in range(8)]
        vt_tiles = [kvp.tile([P, 16 * 65], BF16, name=f"vt{i}") for i in range(4)]
        vt_tiles.append(kvp.tile([2, 16 * 65], BF16, name="vt4"))
        units = []

        def u_load(key, dram):
            def f():
                wcur[key] = load_w(dram, l, F8E5)[8]
            return f

        def u_qt(m):
            def f():
                w3 = wcur["qt"]
                pt = ps_a.tile([P, 512], F32, name="psa")[:, :TO]
                for c in range(4):
                    nc.tensor.matmul(
                        pt, w3[:, 2 * c:2 * c + 2, m * P:(m + 1) * P],
                        xn83[:, 2 * c:2 * c + 2, :],
                        start=(c == 0), stop=(c == 3), perf_mode=DROW)
                rpend.append((rope1(pt, 0, TO, cq), 0, TO, sq, qt[m][:]))
                flush_rope()
            return f

        def u_kt(m, src3d, c0, cw):
            def f():
                w3 = wcur["kt"]
                pt = ps_a.tile([P, 512], F32, name="psa")[:, :cw]
                for c in range(4):
                    nc.tensor.matmul(
                        pt, w3[:, 2 * c:2 * c + 2, m * P:(m + 1) * P],
                        src3d[:, 2 * c:2 * c + 2, :cw],
                        start=(c == 0), stop=(c == 3), perf_mode=DROW)
                rpend.append((rope1(pt, c0, c0 + cw, ck), c0, c0 + cw, sk,
                              kt[m][:, c0:c0 + cw]))
                flush_rope()
            return f

        def u_vt(i, src3d, c0, c1, half):
            def f():
                rows = c1 - c0
                w3 = wcur["vt"]
                vt3 = vt_tiles[i].rearrange("r (h x) -> r h x", x=65)
                pv = ps_a.tile([P, 512], F32, name="psa")[:rows, :]
                # xh stationary: TOP-padded stride makes the pair-step
                # 16B-aligned, so DoubleRow (256-deep fp8) applies
                for c in range(4):
                    nc.tensor.matmul(
                        pv, src3d[:, 2 * c:2 * c + 2, c0:c1],
                        w3[:, 2 * c:2 * c + 2, half * 512:(half + 1) * 512],
                        start=(c == 0), stop=(c == 3), perf_mode=DROW)
                src3 = pv.rearrange("r (h d) -> r h d", d=64)
                nc.scalar.activation(
                    out=vt3[:rows, half * 8:(half + 1) * 8, 0:64], in_=src3,
                    func=AF.Copy)
                if half == 1:
                    nc.vector.memset(vt3[:rows, :, 64:65], 1.0)
            return f

        units.append(u_load("qt", t["wqt"]))
        for m in range(8):
            units.append(u_qt(m))
        units.append(u_load("kt", t["wkt"]))
        for m in range(8):
            for (src3d, c0, cw) in [(xh03, 0, 256), (xh13, 256, TO)]:
                units.append(u_kt(m, src3d, c0, cw))
        units.append(u_load("vt", t["wvt"]))
        for i, (src3d, c0, c1) in enumerate(
                [(xh03, 0, 128), (xh03, 128, 256), (xh13, 0, 128),
                 (xh13, 128, 256), (xh13, 256, TO)]):
            for half in range(2):
                units.append(u_vt(i, src3d, c0, c1, half))
        units.append(lambda: flush_rope(0))

        def cross_filler(n=2):
            for _ in range(n):
                if units:
                    units.pop(0)()

        attention(qm, km, vm,
                  [(i, i * P, (i + 1) * P, 0, TO) for i in range(kp)],
                  osb, osb8, "mem", f"x{l}", filler=cross_filler,
                  clean=kclean)
        while units:
            units.pop(0)()

        # causal-trimmed pieces: key slots [r0A|r0B|r1A|r1B|tail] vs own
        # query cols; slot 0 must span the full query range (start=True).
        # mask multiplies cover only the (mc0, mc1) sub-ranges with zeros.
        attention(qt, kt, vt_tiles,
                  [(0, 0, 128, 0, TO, 0, 128), (1, 128, 256, 128, TO, 128, TO),
                   (2, 256, 384, 0, TO, 0, 128), (3, 384, 512, 128, TO, 128, TO),
                   (4, 512, 514, 256, TO, 256, TO)],
                  osb2, osb28, "self", f"s{l}",
                  groups=[(0, 2), (1, 3), (4,)])

        # ---- FFN/wo weight prefetch: half the FFN tiles are emitted before
        # the wo loads (bufs=8 ring: every transfer can issue by the end of
        # cross-attn, spreading DMA out of the FFN phase). Emitting ALL of
        # them first would deadlock: wom's slot would only free after the
        # FFN consumed w1s0, which needs ln2, which needs wo.
        ffn_w = []

        def ffn_load(sub):
            w1b = w1p.tile([P, 8, 1024], F8E5, name="w1p")
            wdma(w1b[:], chunked(t["w1"][l, :, sub * 1024:(sub + 1) * 1024]))
            w2b = w1p.tile([P, 8, 1024], F8E5, name="w1p")
            wdma(w2b[:], chunked(t["w2"][l, sub * 1024:(sub + 1) * 1024, :]))
            ffn_w.append((w1b, w2b))

        ffn_load(0)
        ffn_load(1)
        wo1 = load_w(t["wom"], l, F8E5)[8]
        wo2 = load_w(t["wot"], l, F8E5)[8]
        ffn_load(2)
        ffn_load(3)

        # ---- combined output projections (DVE: single add into y) ----
        for m in range(8):
            pt = ps_a.tile([P, 512], F32, name="psa")[:, :TO]
            for c in range(4):
                nc.tensor.matmul(
                    pt, wo1[:, 2 * c:2 * c + 2, m * P:(m + 1) * P],
                    osb83[:, 2 * c:2 * c + 2, :],
                    start=(c == 0), stop=False, perf_mode=DROW)
            for c in range(4):
                nc.tensor.matmul(
                    pt, wo2[:, 2 * c:2 * c + 2, m * P:(m + 1) * P],
                    osb283[:, 2 * c:2 * c + 2, :],
                    start=False, stop=(c == 3), perf_mode=DROW)
            nc.vector.tensor_tensor(out=yT[m][:], in0=pt, in1=yT[m][:],
                                    op=ALU.add)

        # ---- FFN (fp8 DoubleRow: h = gelu(xn8 @ W1sub), y += W2sub^T h) ----
        layernorm(yT, xn8, f"ln2_{l}")
        for sub in range(4):
            w1b, w2b = ffn_w[sub]
            hsub = []
            for mc in range(4):
                hp = hbf_p.tile([P, 2, TO], F8E4, name="hbf")
                sp2 = ps_p.tile([P, 2, 512], F32, name="psp")
                for j in range(2):
                    m = 2 * mc + j
                    for c in range(4):
                        nc.tensor.matmul(
                            sp2[:, j, :TO],
                            w1b[:, 2 * c:2 * c + 2, m * P:(m + 1) * P],
                            xn83[:, 2 * c:2 * c + 2, :],
                            start=(c == 0), stop=(c == 3), perf_mode=DROW)
                nc.scalar.activation(out=hp[:], in_=sp2[:, :, :TO],
                                     func=AF.Gelu)
                hsub.append(hp)
            for m in range(8):
                pt = ps_a.tile([P, 512], F32, name="psa")[:, :TO]
                for c in range(4):
                    nc.tensor.matmul(
                        pt, w2b[:, 2 * c:2 * c + 2, m * P:(m + 1) * P],
                        hsub[c][:, :, :],
                        start=(c == 0), stop=(c == 3), perf_mode=DROW)
                nc.vector.tensor_tensor(out=yT[m][:], in0=pt, in1=yT[m][:],
                                        op=ALU.add)

    # ================= head =================
    # logits computed [128-vocab-tile, TO] (vocab on partitions): wout is
    # the DoubleRow stationary, xn8 the moving operand; exp'd tiles are
    # reduced over vocab partitions by an accumulating ones-matmul.
    layernorm(yT, xn, "lnf")
    for k in range(8):
        nc.scalar.activation(out=xn8[k][:], in_=xn[k][:], func=AF.Copy)
    sep = ps_o.tile([1, TO], F32, name="psb_o")
    for nvp in range(8):
        wb = w1p.tile([P, 8, 1024], F8E5, name="w1p")
        wdma(wb[:], chunked(t["wout"][:, nvp * 1024:(nvp + 1) * 1024]))
        for vp in range(4):
            sp2 = ps_p.tile([P, 2, 512], F32, name="psp")
            esc2 = pbf.tile([P, 2, TO], BF16, name="pbf2", bufs=4)
            for j in range(2):
                vsub = vp * 2 + j
                for c in range(4):
                    nc.tensor.matmul(
                        sp2[:, j, :TO],
                        wb[:, 2 * c:2 * c + 2, vsub * P:(vsub + 1) * P],
                        xn83[:, 2 * c:2 * c + 2, :],
                        start=(c == 0), stop=(c == 3), perf_mode=DROW)
            nc.scalar.activation(out=esc2[:], in_=sp2[:, :, :TO], func=AF.Exp)
            for j in range(2):
                vi = nvp * 8 + vp * 2 + j
                nc.tensor.matmul(sep[:], ones_k[:], esc2[:, j, :],
                                 start=(vi == 0), stop=(vi == 63))
    ses = vrow.tile([1, TO], F32, name="vrow")
    nc.scalar.activation(out=ses[:], in_=sep[:], func=AF.Copy)
    dma(t["out_se"][:, :], ses[:])

    # target logit: tl = sum_c wtgt * yf (bf16-rounded, matching logits path)
    tlacc = scr.tile([P, TO], F32, name="tlacc", bufs=1)
    for k in range(8):
        xf = scr.tile([P, TO], F32, name="scr")
        nc.vector.tensor_tensor(out=xf[:], in0=xn[k][:], in1=wtg[k][:],
                                op=ALU.mult)
        if k == 0:
            nc.vector.tensor_copy(out=tlacc[:], in_=xf[:])
        else:
            nc.vector.tensor_add(out=tlacc[:], in0=tlacc[:], in1=xf[:])
    tlp = ps_o.tile([1, TO], F32, name="psb_o")
    nc.tensor.matmul(tlp[:], ones_kf[:], tlacc[:], start=True, stop=True)
    tls = vrow.tile([1, TO], F32, name="vrow")
    nc.scalar.activation(out=tls[:], in_=tlp[:], func=AF.Copy)
    dma(t["out_tl"][:, :], tls[:])


# ======================= host side =======================

def _host_inputs(features, targets, input_lengths, target_lengths, wte, ln1_w,
                 Wq_m, Wk_m, Wv_m, Wo_m, Wq_t, Wk_t, Wv_t, Wo_t, ln2_w, W1,
                 W2, lnf_w, Wout, nlayers):
    bf = ml_dtypes.bfloat16
    f32 = np.float32
    features = np.asarray(features, f32)
    targets = np.asarray(targets).astype(np.int64)
    input_lengths = np.asarray(input_lengths).astype(np.int64)
    target_lengths = np.asarray(target_lengths).astype(np.int64)
    ln1_w = np.asarray(ln1_w, f32)
    ln2_w = np.asarray(ln2_w, f32)
    lnf_w = np.asarray(lnf_w, f32)

    n = targets.shape[0]
    prompt = np.concatenate(
        [np.full((n, 1), STX, np.int64), targets], axis=1)  # [N, T]
    tgt = np.concatenate([targets, np.zeros((n, 1), np.int64)], axis=1)
    tgt[np.arange(n), target_lengths] = ETX

    perm = _rope_perm()
    pos_k = _key_global_idx()
    f8 = 1.0 / np.sqrt(np.float32(D))

    def cast(x):
        return np.ascontiguousarray(np.asarray(x, f32)).astype(bf)

    e4 = ml_dtypes.float8_e4m3
    e5 = ml_dtypes.float8_e5m2

    def cast8(x):
        return np.ascontiguousarray(np.asarray(x, f32)).astype(e5)

    sharedw = {
        "wqm": cast8(np.asarray(Wq_m, f32)[:nlayers] * ln1_w[:nlayers, :, None] * f8),
        "wkm": cast8(np.asarray(Wk_m, f32)[:nlayers]),
        "wvm": cast8(np.asarray(Wv_m, f32)[:nlayers]),
        "wom": cast8(np.asarray(Wo_m, f32)[:nlayers]),
        "wqt": cast8((np.asarray(Wq_t, f32)[:nlayers] * ln1_w[:nlayers, :, None]
                      * f8)[:, :, perm]),
        "wkt": cast8((np.asarray(Wk_t, f32)[:nlayers]
                      * ln1_w[:nlayers, :, None])[:, :, perm]),
        "wvt": cast8(np.asarray(Wv_t, f32)[:nlayers] * ln1_w[:nlayers, :, None]),
        "wot": cast8(np.asarray(Wo_t, f32)[:nlayers]),
        "w1": cast8(np.asarray(W1, f32)[:nlayers] * ln2_w[:nlayers, :, None]),
        "w2": cast8(np.asarray(W2, f32)[:nlayers]),
        "ptsw": _pswap().astype(bf),
    }
    wout_f = np.asarray(Wout, f32) * lnf_w[:, None]
    wout_bf = (np.asarray(Wout, f32) * lnf_w[:, None]).astype(e5)
    sharedw["wout"] = wout_bf

    y0_all = np.asarray(wte, f32)[prompt]  # [N, T, C]

    in_maps, meta = [], []
    for core in range(8):
        nb, r = core // 2, core % 2
        own = _own_global_idx(r)
        ownpos = np.maximum(own, 0)
        y0T = np.where(own[None, :] >= 0, y0_all[nb][ownpos].T, 0.0).astype(bf)
        cosq_, sinq_ = _rope_tables(own)
        cosk_, sink_ = _rope_tables(pos_k)
        mbig, mtail = _self_masks(r)
        memmask = np.where(np.arange(S) < input_lengths[nb], 0.0,
                           NEG).astype(f32)[:, None]
        mm01 = (np.arange(S) < input_lengths[nb]).astype(f32)[:, None]
        padmask = (own >= 0).astype(f32)[None, :]
        wtgt = np.where(own[None, :] >= 0,
                        wout_bf.astype(f32)[:, tgt[nb][ownpos]], 0.0).astype(bf)
        im = {
            "y0": y0T,
            "featT": np.ascontiguousarray(features[nb].T).astype(e4),
            "memmask": memmask,
            "mm01": mm01,
            "mbig": mbig.astype(bf), "mtail": mtail.astype(bf),
            "padmask": padmask,
            "cosq": cosq_.astype(bf), "sinq": sinq_.astype(bf),
            "cosk": cosk_.astype(bf), "sink": sink_.astype(bf),
            "wtgt": wtgt,
        }
        im.update(sharedw)
        in_maps.append(im)
        valid = np.where(own >= 0, (tgt[nb][ownpos] != 0), False)
        meta.append((nb, own, valid))
    return in_maps, meta


def kernel(features, targets, input_lengths, target_lengths, wte, ln1_w,
           Wq_m, Wk_m, Wv_m, Wo_m, Wq_t, Wk_t, Wv_t, Wo_t, ln2_w, W1, W2,
           lnf_w, Wout):
    nlayers = L
    # specialize the program to the live cross-attn key range: keys beyond
    # max(input_lengths) rounded up to 128 are masked for every sample
    kp = int(min(8, max(1, -(-int(np.max(input_lengths)) // 128))))
    kclean = int(min(kp, max(0, int(np.min(input_lengths)) // 128)))
    key = (nlayers, kp, kclean)
    if key not in _prog_cache:
        _prog_cache[key] = _build_program(nlayers, kp, kclean)
    nc = _prog_cache[key]

    in_maps, meta = _host_inputs(
        features, targets, input_lengths, target_lengths, wte, ln1_w,
        Wq_m, Wk_m, Wv_m, Wo_m, Wq_t, Wk_t, Wv_t, Wo_t, ln2_w, W1, W2,
        lnf_w, Wout, nlayers)

    res = run_bass_kernel_spmd(nc, in_maps, core_ids=list(range(8)))
    globals()["LAST_RESULTS"] = res

    num, den = 0.0, 0.0
    for core in range(8):
        r = res.results[core]
        _, own, valid = meta[core]
        sumexp = r["out_se"][0]
        tl = r["out_tl"][0]
        nll = np.log(np.maximum(sumexp, 1e-300)) - tl
        num += float(np.sum(nll[valid]))
        den += float(np.sum(valid))
    return np.float32(num / max(den, 1.0))

